# revision 35
# baseline (speedup 1.0000x reference)
# Trainium2 Bass kernel for nn_MultiHeadTransformer (B=2, S=2048, E=1024, H=16, FF=4096).
#
# Sharding: 8-way tensor/head parallel with ZERO collectives. The reference's
# "faithful raw view" reshape (b, s, 3E) -> (b, 3, H, s, Dh) means q/k/v of head h
# are contiguous 512KB slices of the flat qkv output buffer. Each core computes
# the qkv rows covering exactly the 6 flat blocks (q/k/v x 2 heads) it owns, does
# attention for its 2 heads, and because the inverse raw view maps head h's output
# to token rows [128h, 128(h+1)), the proj/LN/FFN are row-local to the core.
# Per-core offsets within the row-aligned scratch differ mod 3072; they are
# supplied as a tiny uint32 input and applied with one dynamic-offset DMA per
# slot, keeping a single SPMD program.
#
# v2: weight-stationary loops (each weight loaded once, large DMAs), x slots
# pre-transposed on host, proj in bf16, softmax denominator via gpsimd
# partition-broadcast + DVE divide (no DRAM round trips), FFN relu on DVE.
import numpy as np

B, S, E, H, DH, FF = 2, 2048, 1024, 16, 64, 4096
ROW = 3 * E            # 3072 qkv columns
BLK = S * DH           # 131072 elements per (type, head) block
NCORES = 8
P = 128
INV_SCALE = 1.0 / float(np.sqrt(E))

_cached = {}


def _build():
    import concourse.bacc as bacc
    import concourse.bass as bass
    import concourse.mybir as mybir
    import concourse.tile as tile
    from concourse.masks import make_identity

    f32 = mybir.dt.float32
    bf16 = mybir.dt.bfloat16   # attention/QKV path
    fp16 = mybir.dt.float16    # FFN path (finer mantissa for weight casts)
    u32 = mybir.dt.uint32
    AF = mybir.ActivationFunctionType
    ALU = mybir.AluOpType

    nc = bacc.Bacc(trn_type="TRN2", target_bir_lowering=False, debug=False,
                   num_devices=NCORES)

    xq = nc.dram_tensor("xq", [B, 3, P, 8, 88], bf16, kind="ExternalInput").ap()
    xr = nc.dram_tensor("xr", [B, 2, P, E], f32, kind="ExternalInput").ap()
    wqkv = nc.dram_tensor("wqkv", [3, P, 8, 1024], bf16, kind="ExternalInput").ap()
    bqkv = nc.dram_tensor("bqkv", [1, ROW], bf16, kind="ExternalInput").ap()
    wproj = nc.dram_tensor("wproj", [2, P, 8, 512], bf16, kind="ExternalInput").ap()
    bproj = nc.dram_tensor("bproj", [1, E], bf16, kind="ExternalInput").ap()
    w1e = nc.dram_tensor("w1e", [16, P, 2, 8, P], fp16, kind="ExternalInput").ap()
    b1e = nc.dram_tensor("b1e", [1, FF], fp16, kind="ExternalInput").ap()
    w2 = nc.dram_tensor("w2", [2, 4, P, 8, 512], fp16, kind="ExternalInput").ap()
    b2 = nc.dram_tensor("b2", [1, E], fp16, kind="ExternalInput").ap()
    offs = nc.dram_tensor("offs", [1, 4], u32, kind="ExternalInput").ap()
    ones_in = nc.dram_tensor("ones", [P, 130], f32, kind="ExternalInput").ap()
    triu_in = nc.dram_tensor("triu", [P, P], bf16, kind="ExternalInput").ap()
    out = nc.dram_tensor("out", [B, 2, P, E], f32, kind="ExternalOutput").ap()

    mlist = [(b, hh) for b in range(B) for hh in range(2)]

    with tile.TileContext(nc) as tc:
        with tc.tile_pool(name="singles", bufs=1) as singles, \
             tc.tile_pool(name="dram", bufs=1, space="DRAM") as dram:

            ident = singles.tile([P, P], f32)
            make_identity(nc, ident)
            ident_bf = singles.tile([P, P], bf16)
            make_identity(nc, ident_bf)
            # triu_in[k, q] = 0 where q >= k (keep), -30000 where q < k: added
            # to diagonal score tiles pre-exp via a PE matmul so no
            # vector/gpsimd op sits between exp and the AV matmul.
            trin = singles.tile([P, P], bf16)
            nc.sync.dma_start(trin, triu_in)
            eps_t = singles.tile([P, 1], f32)
            nc.vector.memset(eps_t, 1e-5)
            bq_b = singles.tile([1, ROW], bf16)
            nc.sync.dma_start(bq_b, bqkv)
            bp_b = singles.tile([1, E], bf16)
            nc.sync.dma_start(bp_b, bproj)
            b1_b = singles.tile([1, FF], fp16)
            nc.sync.dma_start(b1_b, b1e)
            b2_b = singles.tile([1, E], fp16)
            nc.sync.dma_start(b2_b, b2)
            ones_col = singles.tile([P, 16], f32)
            nc.sync.dma_start(ones_col, ones_in[:, 0:16])
            ones_b = singles.tile([1, P], bf16)
            nc.vector.memset(ones_b, 1.0)
            ones_h = singles.tile([1, P], fp16)
            nc.vector.memset(ones_h, 1.0)
            ones_h512 = singles.tile([1, 512], fp16)
            nc.vector.memset(ones_h512, 1.0)
            # wproj stays resident (16KB/partition) so proj can run per-head,
            # interleaved into the attention phase.
            wp_sb = singles.tile([P, 2, 8, 512], bf16)
            for ns_i in range(2):
                nc.scalar.dma_start(wp_sb[:, ns_i], wproj[ns_i])
            offs_sb = singles.tile([1, 4], u32)
            nc.sync.dma_start(offs_sb, offs)
            off_v = [nc.values_load(offs_sb[:, t:t + 1], min_val=0, max_val=ROW,
                                    skip_runtime_bounds_check=True)
                     for t in range(3)]

            SCR88 = ROW + 88 * ROW
            scr = [[dram.tile([SCR88], bf16, tag=f"scr{b}{t}",
                              name=f"scr{b}_{t}") for t in range(3)]
                   for b in range(B)]

            def transpose_into(pool, dst, src_ap, tag="tp", idt=None, dt_=f32,
                               bufs=None):
                prows = src_ap.shape[0]
                pcols = src_ap.shape[1]
                idt = ident if idt is None else idt
                kw = {} if bufs is None else {"bufs": bufs}
                t_ps = pool.tile([P, P], dt_, tag=tag, name="t_ps", **kw)
                nc.tensor.transpose(t_ps[:pcols, :prows], src_ap,
                                    idt[:prows, :prows])
                nc.vector.tensor_copy(dst, t_ps[:pcols, :prows])

            # ---------------- Phase A: QKV ----------------
            # xT comes pre-transposed from the host; weight-stationary n6 loop
            # loads each wqkv column block exactly once.
            slots = [(b, t) for b in range(B) for t in range(3)]
            with tc.tile_pool(name="qkv_ps", bufs=4, space="PSUM") as qkv_ps, \
                 tc.tile_pool(name="qkv_sb", bufs=1) as qkv_sb, \
                 tc.tile_pool(name="wq_sb", bufs=2) as wq_sb:
                xT = qkv_sb.tile([P, 6, 8, 88], bf16)    # lhsT chunks per slot
                y_sb = qkv_sb.tile([88, 6, ROW], bf16)   # qkv rows per slot
                for m, (b, t) in enumerate(slots):
                    nc.sync.dma_start(xT[:, m], xq[b, t])
                # Weight chunk loaded in two kc-halves so MMs start earlier.
                for n2 in range(3):
                    wh = []
                    for h in range(2):
                        w_sb = wq_sb.tile([P, 4, 1024], bf16, tag=f"wq{h}",
                                          name=f"wq{n2}_{h}")
                        nc.scalar.dma_start(
                            w_sb, wqkv[n2][:, 4 * h:4 * h + 4])
                        wh.append(w_sb)
                    for grp in range(2):
                        ms = [3 * grp, 3 * grp + 1, 3 * grp + 2]
                        acc = [qkv_ps.tile([88, 1024], f32, tag="acc",
                                           name=f"qa{n2}_{m}") for m in ms]
                        for i in range(3):
                            for nh in range(2):
                                ns = slice(n2 * 1024 + nh * 512,
                                           n2 * 1024 + nh * 512 + 512)
                                nc.tensor.matmul(acc[i][:, 512 * nh:
                                                        512 * nh + 512],
                                                 lhsT=ones_b[:, :88],
                                                 rhs=bq_b[:, ns], start=True,
                                                 stop=False)
                        for kc in range(8):
                            for i, m in enumerate(ms):
                                for nh in range(2):
                                    nc.tensor.matmul(
                                        acc[i][:, 512 * nh:512 * nh + 512],
                                        lhsT=xT[:, m, kc, :],
                                        rhs=wh[kc // 4][:, kc % 4,
                                                        512 * nh:512 * nh + 512],
                                        start=False, stop=(kc == 7))
                        for i, m in enumerate(ms):
                            nc.vector.tensor_copy(
                                y_sb[:, m, n2 * 1024:(n2 + 1) * 1024], acc[i])
                for m, (b, t) in enumerate(slots):
                    dst = scr[b][t][bass.ds(off_v[t], 88 * ROW)]
                    nc.sync.dma_start(
                        dst.rearrange("(r c) -> r c", c=ROW), y_sb[:, m, :])

            # -------- Phases B+C (ln spans C..D) --------
            ln_pool_cm = tc.tile_pool(name="ln_pool", bufs=4)
            ln_pool = ln_pool_cm.__enter__()
            outT_cm = tc.tile_pool(name="outT_sb", bufs=1)
            outT_pool = outT_cm.__enter__()

            lnT = ln_pool.tile([P, 8, 4, P], fp16, tag="lnT", bufs=1)
            xr_sb = [ln_pool.tile([P, E], f32, tag="xr", bufs=4,
                                  name=f"xr{mi}") for mi in range(4)]
            for mi, (b, hh) in enumerate(mlist):
                nc.sync.dma_start(xr_sb[mi], xr[b, hh])

            # ---------------- Phase B: attention ----------------
            # q is processed in two 1024-wide halves so each outT accumulator
            # is 2 PSUM banks; bufs=2 lets two (head, half) pipelines overlap.
            with tc.tile_pool(name="at_ps", bufs=2, space="PSUM") as at_ps, \
                 tc.tile_pool(name="ot_ps", bufs=2, space="PSUM") as ot_ps, \
                 tc.tile_pool(name="at_sb", bufs=4) as at_sb, \
                 tc.tile_pool(name="qkv_in", bufs=3) as qkv_in, \
                 tc.tile_pool(name="head_sb", bufs=2) as head_sb:
                for mi, (b, hh) in enumerate(mlist):
                    base = ROW + hh * BLK
                    qT = head_sb.tile([64, S], bf16, tag="qT", name=f"qT{mi}")
                    kT = head_sb.tile([64, S], bf16, tag="kT", name=f"kT{mi}")
                    v_sb = head_sb.tile([P, 16, 65], bf16, tag="v",
                                        name=f"v{mi}")
                    nc.vector.tensor_copy(
                        v_sb[:, :, 64:65],
                        ones_col.rearrange("p (f o) -> p f o", o=1))
                    qn = qkv_in.tile([P, 16, DH], bf16, tag="qn",
                                     name=f"qn{mi}")
                    kn = qkv_in.tile([P, 16, DH], bf16, tag="kn",
                                     name=f"kn{mi}")
                    nc.sync.dma_start(
                        qn, scr[b][0][base:base + BLK]
                        .rearrange("(i p d) -> p i d", p=P, d=DH))
                    nc.sync.dma_start(
                        kn, scr[b][1][base:base + BLK]
                        .rearrange("(i p d) -> p i d", p=P, d=DH))
                    nc.sync.dma_start(
                        v_sb[:, :, 0:64], scr[b][2][base:base + BLK]
                        .rearrange("(i p d) -> p i d", p=P, d=DH))
                    for i in range(16):
                        t_ps = at_ps.tile([P, P], bf16, tag="sc", bufs=3,
                                          name="t_ps")
                        nc.tensor.transpose(t_ps[0:64, :], qn[:, i, :],
                                            ident_bf)
                        nc.vector.tensor_copy(qT[:, i * P:(i + 1) * P],
                                              t_ps[0:64, :])
                        t_ps = at_ps.tile([P, P], bf16, tag="sc", bufs=3,
                                          name="t_ps")
                        nc.tensor.transpose(t_ps[0:64, :], kn[:, i, :],
                                            ident_bf)
                        nc.vector.tensor_copy(kT[:, i * P:(i + 1) * P],
                                              t_ps[0:64, :])
                    oT_sb = outT_pool.tile([64, S], f32, tag="oTsb",
                                           bufs=2, name=f"oTsb{mi}")
                    LAG = 2
                    for gw in range(4):          # 512-wide q windows
                        Ws = 512 * gw
                        npairs = 2 * gw + 2
                        oTw = ot_ps.tile([65, 512], f32, tag="oT",
                                         name=f"oT{mi}_{gw}")
                        pend = []

                        def pop_av(half):
                            # one AV matmul (one half of a pending pair) so
                            # consecutive PE matmuls alternate PSUM banks
                            # (score bank vs oT bank) and fill/drain overlap.
                            if not pend:
                                return
                            p, a_sb, s0, s1 = pend[0]
                            if half == 0:
                                nc.tensor.matmul(
                                    oTw[:, s0:512], lhsT=v_sb[:, 2 * p, :],
                                    rhs=a_sb[:, s0:512],
                                    start=(p == 0), stop=False)
                            else:
                                nc.tensor.matmul(
                                    oTw[:, s1:512], lhsT=v_sb[:, 2 * p + 1, :],
                                    rhs=a_sb[:, 512 + s1:1024],
                                    start=False, stop=(p == npairs - 1))
                                pend.pop(0)

                        for p in range(npairs):
                            j0, j1 = 2 * p, 2 * p + 1
                            s0 = max(128 * j0 - Ws, 0)
                            s1 = max(128 * j1 - Ws, 0)
                            d0 = 128 * j0 >= Ws      # block straddles diagonal
                            d1 = 128 * j1 >= Ws
                            sc_ps = at_ps.tile([P, 1024], f32, tag="sc",
                                               bufs=3, name=f"sc{mi}_{gw}_{p}")
                            a_sb = at_sb.tile([P, 1024], bf16, tag="a",
                                              bufs=6, name=f"a{mi}_{gw}_{p}")
                            ready = len(pend) > LAG
                            nc.tensor.matmul(
                                sc_ps[:, s0:512],
                                lhsT=kT[:, j0 * P:(j0 + 1) * P],
                                rhs=qT[:, Ws + s0:Ws + 512],
                                start=True, stop=not d0)
                            if d0:
                                nc.tensor.matmul(
                                    sc_ps[:, s0:s0 + P], lhsT=ident_bf,
                                    rhs=trin, start=False, stop=True,
                                    skip_group_check=True)
                            if ready:
                                pop_av(0)
                            nc.tensor.matmul(
                                sc_ps[:, 512 + s1:1024],
                                lhsT=kT[:, j1 * P:(j1 + 1) * P],
                                rhs=qT[:, Ws + s1:Ws + 512],
                                start=True, stop=not d1)
                            if d1:
                                nc.tensor.matmul(
                                    sc_ps[:, 512 + s1:512 + s1 + P],
                                    lhsT=ident_bf, rhs=trin,
                                    start=False, stop=True,
                                    skip_group_check=True)
                            if ready:
                                pop_av(1)
                            nc.scalar.activation(a_sb, sc_ps, AF.Exp,
                                                 scale=float(INV_SCALE))
                            pend.append((p, a_sb, s0, s1))
                        while pend:
                            pop_av(0)
                            pop_av(1)
                        # normalize this window; only the partition-broadcast
                        # runs on gpsimd.
                        dn = at_sb.tile([1, 512], f32, tag="dn", bufs=4,
                                        name=f"dn{mi}_{gw}")
                        nc.vector.tensor_copy(dn, oTw[64:65, :])
                        nc.vector.reciprocal_approx_fast(dn, dn)
                        bc = at_sb.tile([64, 512], f32, tag="bc", bufs=4,
                                        name=f"bc{mi}_{gw}")
                        nc.gpsimd.partition_broadcast(bc, dn)
                        nc.vector.tensor_mul(oT_sb[:, Ws:Ws + 512],
                                             oTw[0:64, :], bc)
                    hT = head_sb.tile([P, 8, P], bf16, tag="hT",
                                      name=f"hT{mi}")
                    oT_r = oT_sb.rearrange("d (t a) -> d a t", a=16)
                    for kc in range(8):
                        for ah in range(2):
                            nc.vector.tensor_copy(
                                hT[64 * ah:64 * ah + 64, kc, :],
                                oT_r[:, 2 * kc + ah, :])
                    # ---- proj + residual + LN for this m-tile, interleaved
                    # so the PE gets a dense burst between heads and Phase C
                    # disappears from the critical path.
                    r_sb = at_sb.tile([P, E], f32, tag="r", bufs=2,
                                      name=f"r{mi}")
                    pacc = [ot_ps.tile([P, 512], f32, tag="oT",
                                       name=f"pa{mi}_{ns_i}")
                            for ns_i in range(2)]
                    for ns_i in range(2):
                        nc.tensor.matmul(pacc[ns_i], lhsT=ones_b,
                                         rhs=bp_b[:, 512 * ns_i:
                                                  512 * ns_i + 512],
                                         start=True, stop=False)
                    for kc in range(8):
                        for ns_i in range(2):
                            nc.tensor.matmul(pacc[ns_i], lhsT=hT[:, kc, :],
                                             rhs=wp_sb[:, ns_i, kc, :],
                                             start=False, stop=(kc == 7))
                    for ns_i in range(2):
                        ns = slice(ns_i * 512, (ns_i + 1) * 512)
                        nc.vector.tensor_add(r_sb[:, ns], pacc[ns_i],
                                             xr_sb[mi][:, ns])
                    stats = at_sb.tile([P, 2, 6], f32, tag="stats", bufs=2,
                                       name=f"st{mi}")
                    for sg in range(2):
                        nc.vector.bn_stats(stats[:, sg, :],
                                           r_sb[:, sg * 512:(sg + 1) * 512])
                    mv = at_sb.tile([P, 2], f32, tag="mv", bufs=2,
                                    name=f"mv{mi}")
                    nc.vector.bn_aggr(mv, stats)
                    nc.scalar.activation(mv[:, 1:2], mv[:, 1:2], AF.Sqrt,
                                         bias=eps_t, scale=1.0)
                    nc.vector.reciprocal(mv[:, 1:2], mv[:, 1:2])
                    ln_m = at_sb.tile([P, E], f32, tag="ln", bufs=2,
                                      name=f"ln{mi}")
                    nc.vector.tensor_scalar(
                        ln_m, r_sb, mv[:, 0:1], mv[:, 1:2],
                        ALU.subtract, ALU.mult)
                    for kc in range(8):
                        transpose_into(at_ps, lnT[:, kc, mi, :],
                                       ln_m[:, kc * P:(kc + 1) * P],
                                       tag="sc", bufs=3)

            outT_cm.__exit__(None, None, None)

            # ---------------- Phase D: FFN ----------------
            # h1T computed directly: lhsT = w1 block (e,f), rhs = lnT over all
            # four m-tiles (e, 4*128 tokens) -> h1T (f, tokens). No h1
            # transposes needed, and h1T slices feed w2 as lhsT directly.
            with tc.tile_pool(name="ff_ps", bufs=4, space="PSUM") as ff_ps, \
                 tc.tile_pool(name="fo_ps", bufs=4, space="PSUM") as fo_ps, \
                 tc.tile_pool(name="ff_sb", bufs=1) as ff_sb, \
                 tc.tile_pool(name="wf_sb", bufs=3) as wf_sb, \
                 tc.tile_pool(name="w2_sb", bufs=2) as w2_sb, \
                 tc.tile_pool(name="o_sb", bufs=2) as o_pool:
                h1T = ff_sb.tile([P, 32, 4, P], fp16)   # (f-part, fc, token)
                for fcp in range(16):
                    w_sb = wf_sb.tile([P, 2, 8, P], fp16, tag="w1s",
                                      name=f"w1_{fcp}")
                    nc.scalar.dma_start(w_sb, w1e[fcp])
                    facc = [ff_ps.tile([P, 512], f32, tag="facc",
                                       name=f"fa{2 * fcp + fl}")
                            for fl in range(2)]
                    for fl in range(2):
                        fc = 2 * fcp + fl
                        nc.tensor.matmul(
                            facc[fl], lhsT=b1_b[:, 128 * fc:128 * (fc + 1)],
                            rhs=ones_h512, start=True, stop=False)
                    for kc in range(8):
                        for fl in range(2):
                            nc.tensor.matmul(
                                facc[fl], lhsT=w_sb[:, fl, kc, :],
                                rhs=lnT[:, kc, :, :],
                                start=False, stop=(kc == 7))
                    for fl in range(2):
                        nc.vector.tensor_relu(h1T[:, 2 * fcp + fl, :, :],
                                              facc[fl])
                for ns_i in range(2):
                    ns = slice(ns_i * 512, (ns_i + 1) * 512)
                    acc = [fo_ps.tile([P, 512], f32, tag="oacc",
                                      name=f"oa{ns_i}_{mi}") for mi in range(4)]
                    for mi in range(4):
                        nc.tensor.matmul(acc[mi], lhsT=ones_h,
                                         rhs=b2_b[:, ns],
                                         start=True, stop=False)
                    for kb in range(4):
                        w_sb = w2_sb.tile([P, 8, 512], fp16, tag="w2s",
                                          name=f"w2_{ns_i}_{kb}")
                        nc.scalar.dma_start(w_sb, w2[ns_i, kb])
                        for kc8 in range(8):
                            kc = 8 * kb + kc8
                            for mi in range(4):
                                nc.tensor.matmul(acc[mi],
                                                 lhsT=h1T[:, kc, mi, :],
                                                 rhs=w_sb[:, kc8, :],
                                                 start=False, stop=(kc == 31))
                    for mi, (b, hh) in enumerate(mlist):
                        o_sb = o_pool.tile([P, 512], f32, tag="o",
                                           name=f"o{ns_i}_{mi}")
                        nc.vector.tensor_copy(o_sb, acc[mi])
                        nc.sync.dma_start(out[b, hh, :, ns], o_sb)

            ln_pool_cm.__exit__(None, None, None)

    nc.compile()
    return nc


def _get_nc():
    if "nc" not in _cached:
        _cached["nc"] = _build()
    return _cached["nc"]


def _make_in_maps(inputs):
    import ml_dtypes
    x = np.ascontiguousarray(np.asarray(inputs["x"], dtype=np.float32))
    w_qkv = np.ascontiguousarray(np.asarray(inputs["w_qkv"], dtype=np.float32))
    b_qkv = np.asarray(inputs["b_qkv"], dtype=np.float32).reshape(1, ROW)
    w_proj = np.ascontiguousarray(np.asarray(inputs["w_proj"], dtype=np.float32))
    b_proj = np.asarray(inputs["b_proj"], dtype=np.float32).reshape(1, E)
    ln_g = np.asarray(inputs["ln_g"], dtype=np.float32)
    ln_b = np.asarray(inputs["ln_b"], dtype=np.float32)
    w1 = np.asarray(inputs["w1"], dtype=np.float32)
    b1 = np.asarray(inputs["b1"], dtype=np.float32)
    w2 = np.ascontiguousarray(np.asarray(inputs["w2"], dtype=np.float32))
    b2 = np.asarray(inputs["b2"], dtype=np.float32).reshape(1, E)

    w1e = (ln_g[:, None] * w1).astype(np.float32)
    b1e = (b1 + ln_b @ w1).reshape(1, FF).astype(np.float32)

    # wqkv [3, 128, 8, 1024]: [n2, p, kc, f] = w_qkv[kc*128+p, n2*1024+f]
    w_qkv_t = np.ascontiguousarray(
        w_qkv.reshape(8, P, 3, 1024).transpose(2, 1, 0, 3)
    ).astype(ml_dtypes.bfloat16)
    # wproj [2, 128, 8, 512]
    w_proj_t = np.ascontiguousarray(
        w_proj.reshape(8, P, 2, 512).transpose(2, 1, 0, 3)
    ).astype(ml_dtypes.bfloat16)
    # w1e [16, 128, 2, 8, 128]: [fcp, p, fl, kc, f] = w1e[kc*128+p, (2*fcp+fl)*128+f]
    w1e_t = np.ascontiguousarray(
        w1e.reshape(8, P, 16, 2, P).transpose(2, 1, 3, 0, 4)
    ).astype(np.float16)
    # w2 [2, 4, 128, 8, 512]: [ns, kb, p, kc8, f] = w2[(kb*8+kc8)*128+p, ns*512+f]
    w2_t = np.ascontiguousarray(
        w2.reshape(4, 8, P, 2, 512).transpose(3, 0, 2, 1, 4)
    ).astype(np.float16)

    ones_host = np.ones((P, 130), np.float32)
    # additive causal mask for diagonal tiles: 0 where q >= k, -30000 where
    # q < k (exp(-30000/32) == 0)
    triu_host = ((np.triu(np.ones((P, P))) - 1.0) *
                 30000.0).astype(ml_dtypes.bfloat16)
    b_qkv_b = b_qkv.astype(ml_dtypes.bfloat16)
    b_proj_b = b_proj.astype(ml_dtypes.bfloat16)
    b1e_h = b1e.astype(np.float16)
    b2_h = b2.astype(np.float16)
    in_maps = []
    for c in range(NCORES):
        # xq [B, 3, 128, 8, 88]: pre-transposed x rows per slot:
        # [b, t, p, kc, r] = x[b, T0+r, kc*128+p]
        xq = np.zeros((B, 3, P, 8, 88), ml_dtypes.bfloat16)
        offs = np.zeros((1, 4), np.uint32)
        for t in range(3):
            start = (16 * t + 2 * c) * BLK
            T0 = start // ROW
            offs[0, t] = ROW - (start - T0 * ROW)
            n = min(88, S - T0)
            for b in range(B):
                xt = x[b, T0:T0 + n].T.reshape(8, P, n)   # [kc, p, r]
                xq[b, t, :, :, :n] = xt.transpose(1, 0, 2)
        xr = np.zeros((B, 2, P, E), np.float32)
        for hh in range(2):
            h_ = 2 * c + hh
            for b in range(B):
                xr[b, hh] = x[b, P * h_:P * (h_ + 1)]
        in_maps.append({
            "xq": xq, "xr": xr, "offs": offs,
            "ones": ones_host, "triu": triu_host,
            "wqkv": w_qkv_t, "bqkv": b_qkv_b, "wproj": w_proj_t,
            "bproj": b_proj_b,
            "w1e": w1e_t, "b1e": b1e_h, "w2": w2_t, "b2": b2_h,
        })
    return in_maps


def _run(inputs, trace=False, trace_cores=None):
    import sys
    if "/opt/trn_rl_repo" not in sys.path:
        sys.path.insert(0, "/opt/trn_rl_repo")
    from concourse.bass_utils import run_bass_kernel_spmd
    nc = _get_nc()
    in_maps = _make_in_maps(inputs)
    kwargs = {}
    if trace:
        kwargs["trace"] = True
        if trace_cores is not None:
            kwargs["trace_cores"] = trace_cores
    res = run_bass_kernel_spmd(nc, in_maps, list(range(NCORES)), **kwargs)
    full = np.zeros((B, S, E), np.float32)
    for c in range(NCORES):
        o = res.results[c]["out"]
        for hh in range(2):
            h_ = 2 * c + hh
            for b in range(B):
                full[b, P * h_:P * (h_ + 1)] = o[b, hh]
    return full, res


def kernel(**inputs) -> np.ndarray:
    import sys
    if "/opt/trn_rl_repo" not in sys.path:
        sys.path.insert(0, "/opt/trn_rl_repo")
    full, _ = _run(inputs)
    return full


# revision 39
# speedup vs baseline: 1.0334x; 1.0334x over previous
# Trainium2 Bass kernel for nn_MultiHeadTransformer (B=2, S=2048, E=1024, H=16, FF=4096).
#
# Sharding: 8-way tensor/head parallel with ZERO collectives. The reference's
# "faithful raw view" reshape (b, s, 3E) -> (b, 3, H, s, Dh) means q/k/v of head h
# are contiguous 512KB slices of the flat qkv output buffer. Each core computes
# the qkv rows covering exactly the 6 flat blocks (q/k/v x 2 heads) it owns, does
# attention for its 2 heads, and because the inverse raw view maps head h's output
# to token rows [128h, 128(h+1)), the proj/LN/FFN are row-local to the core.
# Per-core offsets within the row-aligned scratch differ mod 3072; they are
# supplied as a tiny uint32 input and applied with one dynamic-offset DMA per
# slot, keeping a single SPMD program.
#
# v2: weight-stationary loops (each weight loaded once, large DMAs), x slots
# pre-transposed on host, proj in bf16, softmax denominator via gpsimd
# partition-broadcast + DVE divide (no DRAM round trips), FFN relu on DVE.
import numpy as np

B, S, E, H, DH, FF = 2, 2048, 1024, 16, 64, 4096
ROW = 3 * E            # 3072 qkv columns
BLK = S * DH           # 131072 elements per (type, head) block
NCORES = 8
P = 128
INV_SCALE = 1.0 / float(np.sqrt(E))

_cached = {}


def _build():
    import concourse.bacc as bacc
    import concourse.bass as bass
    import concourse.mybir as mybir
    import concourse.tile as tile
    from concourse.masks import make_identity

    f32 = mybir.dt.float32
    bf16 = mybir.dt.bfloat16   # attention/QKV path
    fp16 = mybir.dt.float16    # FFN path (finer mantissa for weight casts)
    u32 = mybir.dt.uint32
    AF = mybir.ActivationFunctionType
    ALU = mybir.AluOpType

    nc = bacc.Bacc(trn_type="TRN2", target_bir_lowering=False, debug=False,
                   num_devices=NCORES)

    xq = nc.dram_tensor("xq", [B, 3, P, 8, 88], bf16, kind="ExternalInput").ap()
    xr = nc.dram_tensor("xr", [B, 2, P, E], f32, kind="ExternalInput").ap()
    wqkv = nc.dram_tensor("wqkv", [3, P, 8, 1024], bf16, kind="ExternalInput").ap()
    bqkv = nc.dram_tensor("bqkv", [1, ROW], bf16, kind="ExternalInput").ap()
    wproj = nc.dram_tensor("wproj", [2, P, 8, 512], bf16, kind="ExternalInput").ap()
    bproj = nc.dram_tensor("bproj", [1, E], bf16, kind="ExternalInput").ap()
    w1e = nc.dram_tensor("w1e", [16, P, 2, 8, P], fp16, kind="ExternalInput").ap()
    b1e = nc.dram_tensor("b1e", [1, FF], fp16, kind="ExternalInput").ap()
    w2 = nc.dram_tensor("w2", [2, 4, P, 8, 512], fp16, kind="ExternalInput").ap()
    b2 = nc.dram_tensor("b2", [1, E], fp16, kind="ExternalInput").ap()
    offs = nc.dram_tensor("offs", [1, 4], u32, kind="ExternalInput").ap()
    ones_in = nc.dram_tensor("ones", [P, 130], f32, kind="ExternalInput").ap()
    triu_in = nc.dram_tensor("triu", [P, P], bf16, kind="ExternalInput").ap()
    out = nc.dram_tensor("out", [B, 2, P, E], f32, kind="ExternalOutput").ap()

    mlist = [(b, hh) for b in range(B) for hh in range(2)]

    with tile.TileContext(nc) as tc:
        with tc.tile_pool(name="singles", bufs=1) as singles, \
             tc.tile_pool(name="dram", bufs=1, space="DRAM") as dram:

            ident = singles.tile([P, P], f32)
            make_identity(nc, ident)
            ident_bf = singles.tile([P, P], bf16)
            make_identity(nc, ident_bf)
            # triu_in[k, q] = 0 where q >= k (keep), -30000 where q < k: added
            # to diagonal score tiles pre-exp via a PE matmul so no
            # vector/gpsimd op sits between exp and the AV matmul.
            trin = singles.tile([P, P], bf16)
            nc.sync.dma_start(trin, triu_in)
            eps_t = singles.tile([P, 1], f32)
            nc.vector.memset(eps_t, 1e-5)
            bq_b = singles.tile([1, ROW], bf16)
            nc.sync.dma_start(bq_b, bqkv)
            bp_b = singles.tile([1, E], bf16)
            nc.sync.dma_start(bp_b, bproj)
            b1_b = singles.tile([1, FF], fp16)
            nc.sync.dma_start(b1_b, b1e)
            b2_b = singles.tile([1, E], fp16)
            nc.sync.dma_start(b2_b, b2)
            ones_col = singles.tile([P, 16], f32)
            nc.sync.dma_start(ones_col, ones_in[:, 0:16])
            ones_b = singles.tile([1, P], bf16)
            nc.vector.memset(ones_b, 1.0)
            ones_h = singles.tile([1, P], fp16)
            nc.vector.memset(ones_h, 1.0)
            ones_h512 = singles.tile([1, 512], fp16)
            nc.vector.memset(ones_h512, 1.0)
            # wproj stays resident (16KB/partition) so proj can run per-head,
            # interleaved into the attention phase.
            wp_sb = singles.tile([P, 2, 8, 512], bf16)
            for ns_i in range(2):
                nc.scalar.dma_start(wp_sb[:, ns_i], wproj[ns_i])
            offs_sb = singles.tile([1, 4], u32)
            nc.sync.dma_start(offs_sb, offs)
            off_v = [nc.values_load(offs_sb[:, t:t + 1], min_val=0, max_val=ROW,
                                    skip_runtime_bounds_check=True)
                     for t in range(3)]

            SCR88 = ROW + 88 * ROW
            scr = [[dram.tile([SCR88], bf16, tag=f"scr{b}{t}",
                              name=f"scr{b}_{t}") for t in range(3)]
                   for b in range(B)]

            def transpose_into(pool, dst, src_ap, tag="tp", idt=None, dt_=f32,
                               bufs=None):
                prows = src_ap.shape[0]
                pcols = src_ap.shape[1]
                idt = ident if idt is None else idt
                kw = {} if bufs is None else {"bufs": bufs}
                t_ps = pool.tile([P, P], dt_, tag=tag, name="t_ps", **kw)
                nc.tensor.transpose(t_ps[:pcols, :prows], src_ap,
                                    idt[:prows, :prows])
                nc.vector.tensor_copy(dst, t_ps[:pcols, :prows])

            # ---------------- Phase A: QKV ----------------
            # xT comes pre-transposed from the host; weight-stationary n6 loop
            # loads each wqkv column block exactly once.
            slots = [(b, t) for b in range(B) for t in range(3)]
            with tc.tile_pool(name="qkv_ps", bufs=4, space="PSUM") as qkv_ps, \
                 tc.tile_pool(name="qkv_sb", bufs=1) as qkv_sb, \
                 tc.tile_pool(name="wq_sb", bufs=2) as wq_sb:
                xT = qkv_sb.tile([P, 6, 8, 88], bf16)    # lhsT chunks per slot
                y_sb = qkv_sb.tile([88, 6, ROW], bf16)   # qkv rows per slot
                for m, (b, t) in enumerate(slots):
                    nc.sync.dma_start(xT[:, m], xq[b, t])
                # Weight chunk loaded in two kc-halves so MMs start earlier.
                for n2 in range(3):
                    wh = []
                    for h in range(2):
                        w_sb = wq_sb.tile([P, 4, 1024], bf16, tag=f"wq{h}",
                                          name=f"wq{n2}_{h}")
                        nc.scalar.dma_start(
                            w_sb, wqkv[n2][:, 4 * h:4 * h + 4])
                        wh.append(w_sb)
                    for grp in range(2):
                        ms = [3 * grp, 3 * grp + 1, 3 * grp + 2]
                        acc = [qkv_ps.tile([88, 1024], f32, tag="acc",
                                           name=f"qa{n2}_{m}") for m in ms]
                        for i in range(3):
                            for nh in range(2):
                                ns = slice(n2 * 1024 + nh * 512,
                                           n2 * 1024 + nh * 512 + 512)
                                nc.tensor.matmul(acc[i][:, 512 * nh:
                                                        512 * nh + 512],
                                                 lhsT=ones_b[:, :88],
                                                 rhs=bq_b[:, ns], start=True,
                                                 stop=False)
                        for kc in range(8):
                            for i, m in enumerate(ms):
                                for nh in range(2):
                                    nc.tensor.matmul(
                                        acc[i][:, 512 * nh:512 * nh + 512],
                                        lhsT=xT[:, m, kc, :],
                                        rhs=wh[kc // 4][:, kc % 4,
                                                        512 * nh:512 * nh + 512],
                                        start=False, stop=(kc == 7))
                        for i, m in enumerate(ms):
                            nc.vector.tensor_copy(
                                y_sb[:, m, n2 * 1024:(n2 + 1) * 1024], acc[i])
                for m, (b, t) in enumerate(slots):
                    dst = scr[b][t][bass.ds(off_v[t], 88 * ROW)]
                    nc.sync.dma_start(
                        dst.rearrange("(r c) -> r c", c=ROW), y_sb[:, m, :])

            # -------- Phases B+C (ln spans C..D) --------
            ln_pool_cm = tc.tile_pool(name="ln_pool", bufs=4)
            ln_pool = ln_pool_cm.__enter__()
            outT_cm = tc.tile_pool(name="outT_sb", bufs=1)
            outT_pool = outT_cm.__enter__()

            lnT = ln_pool.tile([P, 8, 4, P], fp16, tag="lnT", bufs=1)
            xr_sb = [ln_pool.tile([P, E], f32, tag="xr", bufs=4,
                                  name=f"xr{mi}") for mi in range(4)]
            for mi, (b, hh) in enumerate(mlist):
                nc.sync.dma_start(xr_sb[mi], xr[b, hh])

            # ---------------- Phase B: attention ----------------
            # q is processed in two 1024-wide halves so each outT accumulator
            # is 2 PSUM banks; bufs=2 lets two (head, half) pipelines overlap.
            with tc.tile_pool(name="at_ps", bufs=2, space="PSUM") as at_ps, \
                 tc.tile_pool(name="ot_ps", bufs=2, space="PSUM") as ot_ps, \
                 tc.tile_pool(name="at_sb", bufs=4) as at_sb, \
                 tc.tile_pool(name="qkv_in", bufs=3) as qkv_in, \
                 tc.tile_pool(name="head_sb", bufs=2) as head_sb:
                for mi, (b, hh) in enumerate(mlist):
                    base = ROW + hh * BLK
                    # qTb: transposed q duplicated in both partition halves;
                    # kTd: even k-blocks in partitions 0-63, odd in 64-127.
                    # The two score matmuls of a block pair then use different
                    # PE row-groups, so the second LDWEIGHTS pulls ahead
                    # during the first matmul.
                    qTb = head_sb.tile([P, S], bf16, tag="qT", name=f"qT{mi}")
                    kTd = head_sb.tile([P, S // 2], bf16, tag="kT",
                                       name=f"kT{mi}")
                    v_sb = head_sb.tile([P, 16, 65], bf16, tag="v",
                                        name=f"v{mi}")
                    nc.vector.tensor_copy(
                        v_sb[:, :, 64:65],
                        ones_col.rearrange("p (f o) -> p f o", o=1))
                    qn = qkv_in.tile([P, 16, DH], bf16, tag="qn",
                                     name=f"qn{mi}")
                    kn = qkv_in.tile([P, 16, DH], bf16, tag="kn",
                                     name=f"kn{mi}")
                    nc.sync.dma_start(
                        qn, scr[b][0][base:base + BLK]
                        .rearrange("(i p d) -> p i d", p=P, d=DH))
                    nc.sync.dma_start(
                        kn, scr[b][1][base:base + BLK]
                        .rearrange("(i p d) -> p i d", p=P, d=DH))
                    nc.sync.dma_start(
                        v_sb[:, :, 0:64], scr[b][2][base:base + BLK]
                        .rearrange("(i p d) -> p i d", p=P, d=DH))
                    for i in range(16):
                        t_ps = at_ps.tile([P, P], bf16, tag="sc", bufs=3,
                                          name="t_ps")
                        nc.tensor.transpose(t_ps[0:64, :], qn[:, i, :],
                                            ident_bf)
                        nc.tensor.transpose(t_ps[64:128, :], qn[:, i, :],
                                            ident_bf)
                        nc.vector.tensor_copy(qTb[:, i * P:(i + 1) * P], t_ps)
                    for p8 in range(8):
                        t_ps = at_ps.tile([P, P], bf16, tag="sc", bufs=3,
                                          name="t_ps")
                        nc.tensor.transpose(t_ps[0:64, :], kn[:, 2 * p8, :],
                                            ident_bf)
                        nc.tensor.transpose(t_ps[64:128, :],
                                            kn[:, 2 * p8 + 1, :], ident_bf)
                        nc.vector.tensor_copy(kTd[:, p8 * P:(p8 + 1) * P],
                                              t_ps)
                    oT_sb = outT_pool.tile([64, S], f32, tag="oTsb",
                                           bufs=2, name=f"oTsb{mi}")
                    LAG = 2
                    for gw in range(4):          # 512-wide q windows
                        Ws = 512 * gw
                        npairs = 2 * gw + 2
                        oTw = ot_ps.tile([65, 512], f32, tag="oT",
                                         name=f"oT{mi}_{gw}")
                        pend = []

                        def pop_av(half):
                            # one AV matmul (one half of a pending pair) so
                            # consecutive PE matmuls alternate PSUM banks
                            # (score bank vs oT bank) and fill/drain overlap.
                            if not pend:
                                return
                            p, a_sb, s0, s1 = pend[0]
                            if half == 0:
                                nc.tensor.matmul(
                                    oTw[:, s0:512], lhsT=v_sb[:, 2 * p, :],
                                    rhs=a_sb[:, s0:512],
                                    start=(p == 0), stop=False)
                            else:
                                nc.tensor.matmul(
                                    oTw[:, s1:512], lhsT=v_sb[:, 2 * p + 1, :],
                                    rhs=a_sb[:, 512 + s1:1024],
                                    start=False, stop=(p == npairs - 1))
                                pend.pop(0)

                        for p in range(npairs):
                            j0, j1 = 2 * p, 2 * p + 1
                            s0 = max(128 * j0 - Ws, 0)
                            s1 = max(128 * j1 - Ws, 0)
                            d0 = 128 * j0 >= Ws      # block straddles diagonal
                            d1 = 128 * j1 >= Ws
                            sc_ps = at_ps.tile([P, 1024], f32, tag="sc",
                                               bufs=3, name=f"sc{mi}_{gw}_{p}")
                            a_sb = at_sb.tile([P, 1024], bf16, tag="a",
                                              bufs=6, name=f"a{mi}_{gw}_{p}")
                            ready = len(pend) > LAG
                            nc.tensor.matmul(
                                sc_ps[:, s0:512],
                                lhsT=kTd[0:64, p * P:(p + 1) * P],
                                rhs=qTb[0:64, Ws + s0:Ws + 512],
                                start=True, stop=not d0)
                            if d0:
                                nc.tensor.matmul(
                                    sc_ps[:, s0:s0 + P], lhsT=ident_bf,
                                    rhs=trin, start=False, stop=True,
                                    skip_group_check=True)
                            if ready:
                                pop_av(0)
                            nc.tensor.matmul(
                                sc_ps[:, 512 + s1:1024],
                                lhsT=kTd[64:128, p * P:(p + 1) * P],
                                rhs=qTb[64:128, Ws + s1:Ws + 512],
                                start=True, stop=not d1)
                            if d1:
                                nc.tensor.matmul(
                                    sc_ps[:, 512 + s1:512 + s1 + P],
                                    lhsT=ident_bf, rhs=trin,
                                    start=False, stop=True,
                                    skip_group_check=True)
                            if ready:
                                pop_av(1)
                            nc.scalar.activation(a_sb, sc_ps, AF.Exp,
                                                 scale=float(INV_SCALE))
                            pend.append((p, a_sb, s0, s1))
                        while pend:
                            pop_av(0)
                            pop_av(1)
                        # normalize this window; only the partition-broadcast
                        # runs on gpsimd.
                        dn = at_sb.tile([1, 512], f32, tag="dn", bufs=4,
                                        name=f"dn{mi}_{gw}")
                        nc.vector.tensor_copy(dn, oTw[64:65, :])
                        nc.vector.reciprocal_approx_fast(dn, dn)
                        bc = at_sb.tile([64, 512], f32, tag="bc", bufs=4,
                                        name=f"bc{mi}_{gw}")
                        nc.gpsimd.partition_broadcast(bc, dn)
                        nc.vector.tensor_mul(oT_sb[:, Ws:Ws + 512],
                                             oTw[0:64, :], bc)
                    hT = head_sb.tile([P, 8, P], bf16, tag="hT",
                                      name=f"hT{mi}")
                    oT_r = oT_sb.rearrange("d (t a) -> d a t", a=16)
                    for kc in range(8):
                        for ah in range(2):
                            nc.vector.tensor_copy(
                                hT[64 * ah:64 * ah + 64, kc, :],
                                oT_r[:, 2 * kc + ah, :])
                    # ---- proj + residual + LN for this m-tile, interleaved
                    # so the PE gets a dense burst between heads and Phase C
                    # disappears from the critical path.
                    r_sb = at_sb.tile([P, E], f32, tag="r", bufs=2,
                                      name=f"r{mi}")
                    pacc = [ot_ps.tile([P, 512], f32, tag="oT",
                                       name=f"pa{mi}_{ns_i}")
                            for ns_i in range(2)]
                    for ns_i in range(2):
                        nc.tensor.matmul(pacc[ns_i], lhsT=ones_b,
                                         rhs=bp_b[:, 512 * ns_i:
                                                  512 * ns_i + 512],
                                         start=True, stop=False)
                    for kc in range(8):
                        for ns_i in range(2):
                            nc.tensor.matmul(pacc[ns_i], lhsT=hT[:, kc, :],
                                             rhs=wp_sb[:, ns_i, kc, :],
                                             start=False, stop=(kc == 7))
                    for ns_i in range(2):
                        ns = slice(ns_i * 512, (ns_i + 1) * 512)
                        nc.vector.tensor_add(r_sb[:, ns], pacc[ns_i],
                                             xr_sb[mi][:, ns])
                    stats = at_sb.tile([P, 2, 6], f32, tag="stats", bufs=2,
                                       name=f"st{mi}")
                    for sg in range(2):
                        nc.vector.bn_stats(stats[:, sg, :],
                                           r_sb[:, sg * 512:(sg + 1) * 512])
                    mv = at_sb.tile([P, 2], f32, tag="mv", bufs=2,
                                    name=f"mv{mi}")
                    nc.vector.bn_aggr(mv, stats)
                    nc.scalar.activation(mv[:, 1:2], mv[:, 1:2], AF.Sqrt,
                                         bias=eps_t, scale=1.0)
                    nc.vector.reciprocal(mv[:, 1:2], mv[:, 1:2])
                    ln_m = at_sb.tile([P, E], f32, tag="ln", bufs=2,
                                      name=f"ln{mi}")
                    nc.vector.tensor_scalar(
                        ln_m, r_sb, mv[:, 0:1], mv[:, 1:2],
                        ALU.subtract, ALU.mult)
                    for kc in range(8):
                        transpose_into(at_ps, lnT[:, kc, mi, :],
                                       ln_m[:, kc * P:(kc + 1) * P],
                                       tag="sc", bufs=3)

            outT_cm.__exit__(None, None, None)

            # ---------------- Phase D: FFN ----------------
            # h1T computed directly: lhsT = w1 block (e,f), rhs = lnT over all
            # four m-tiles (e, 4*128 tokens) -> h1T (f, tokens). No h1
            # transposes needed, and h1T slices feed w2 as lhsT directly.
            with tc.tile_pool(name="ff_ps", bufs=4, space="PSUM") as ff_ps, \
                 tc.tile_pool(name="fo_ps", bufs=4, space="PSUM") as fo_ps, \
                 tc.tile_pool(name="ff_sb", bufs=1) as ff_sb, \
                 tc.tile_pool(name="wf_sb", bufs=3) as wf_sb, \
                 tc.tile_pool(name="w2_sb", bufs=2) as w2_sb, \
                 tc.tile_pool(name="o_sb", bufs=2) as o_pool:
                h1T = ff_sb.tile([P, 32, 4, P], fp16)   # (f-part, fc, token)
                for fcp in range(16):
                    w_sb = wf_sb.tile([P, 2, 8, P], fp16, tag="w1s",
                                      name=f"w1_{fcp}")
                    nc.scalar.dma_start(w_sb, w1e[fcp])
                    facc = [ff_ps.tile([P, 512], f32, tag="facc",
                                       name=f"fa{2 * fcp + fl}")
                            for fl in range(2)]
                    for fl in range(2):
                        fc = 2 * fcp + fl
                        nc.tensor.matmul(
                            facc[fl], lhsT=b1_b[:, 128 * fc:128 * (fc + 1)],
                            rhs=ones_h512, start=True, stop=False)
                    for kc in range(8):
                        for fl in range(2):
                            nc.tensor.matmul(
                                facc[fl], lhsT=w_sb[:, fl, kc, :],
                                rhs=lnT[:, kc, :, :],
                                start=False, stop=(kc == 7))
                    for fl in range(2):
                        nc.vector.tensor_relu(h1T[:, 2 * fcp + fl, :, :],
                                              facc[fl])
                for ns_i in range(2):
                    ns = slice(ns_i * 512, (ns_i + 1) * 512)
                    acc = [fo_ps.tile([P, 512], f32, tag="oacc",
                                      name=f"oa{ns_i}_{mi}") for mi in range(4)]
                    for mi in range(4):
                        nc.tensor.matmul(acc[mi], lhsT=ones_h,
                                         rhs=b2_b[:, ns],
                                         start=True, stop=False)
                    for kb in range(4):
                        w_sb = w2_sb.tile([P, 8, 512], fp16, tag="w2s",
                                          name=f"w2_{ns_i}_{kb}")
                        nc.scalar.dma_start(w_sb, w2[ns_i, kb])
                        for kc8 in range(8):
                            kc = 8 * kb + kc8
                            for mi in range(4):
                                nc.tensor.matmul(acc[mi],
                                                 lhsT=h1T[:, kc, mi, :],
                                                 rhs=w_sb[:, kc8, :],
                                                 start=False, stop=(kc == 31))
                    for mi, (b, hh) in enumerate(mlist):
                        o_sb = o_pool.tile([P, 512], f32, tag="o",
                                           name=f"o{ns_i}_{mi}")
                        nc.vector.tensor_copy(o_sb, acc[mi])
                        nc.sync.dma_start(out[b, hh, :, ns], o_sb)

            ln_pool_cm.__exit__(None, None, None)

    nc.compile()
    return nc


def _get_nc():
    if "nc" not in _cached:
        _cached["nc"] = _build()
    return _cached["nc"]


def _make_in_maps(inputs):
    import ml_dtypes
    x = np.ascontiguousarray(np.asarray(inputs["x"], dtype=np.float32))
    w_qkv = np.ascontiguousarray(np.asarray(inputs["w_qkv"], dtype=np.float32))
    b_qkv = np.asarray(inputs["b_qkv"], dtype=np.float32).reshape(1, ROW)
    w_proj = np.ascontiguousarray(np.asarray(inputs["w_proj"], dtype=np.float32))
    b_proj = np.asarray(inputs["b_proj"], dtype=np.float32).reshape(1, E)
    ln_g = np.asarray(inputs["ln_g"], dtype=np.float32)
    ln_b = np.asarray(inputs["ln_b"], dtype=np.float32)
    w1 = np.asarray(inputs["w1"], dtype=np.float32)
    b1 = np.asarray(inputs["b1"], dtype=np.float32)
    w2 = np.ascontiguousarray(np.asarray(inputs["w2"], dtype=np.float32))
    b2 = np.asarray(inputs["b2"], dtype=np.float32).reshape(1, E)

    w1e = (ln_g[:, None] * w1).astype(np.float32)
    b1e = (b1 + ln_b @ w1).reshape(1, FF).astype(np.float32)

    # wqkv [3, 128, 8, 1024]: [n2, p, kc, f] = w_qkv[kc*128+p, n2*1024+f]
    w_qkv_t = np.ascontiguousarray(
        w_qkv.reshape(8, P, 3, 1024).transpose(2, 1, 0, 3)
    ).astype(ml_dtypes.bfloat16)
    # wproj [2, 128, 8, 512]
    w_proj_t = np.ascontiguousarray(
        w_proj.reshape(8, P, 2, 512).transpose(2, 1, 0, 3)
    ).astype(ml_dtypes.bfloat16)
    # w1e [16, 128, 2, 8, 128]: [fcp, p, fl, kc, f] = w1e[kc*128+p, (2*fcp+fl)*128+f]
    w1e_t = np.ascontiguousarray(
        w1e.reshape(8, P, 16, 2, P).transpose(2, 1, 3, 0, 4)
    ).astype(np.float16)
    # w2 [2, 4, 128, 8, 512]: [ns, kb, p, kc8, f] = w2[(kb*8+kc8)*128+p, ns*512+f]
    w2_t = np.ascontiguousarray(
        w2.reshape(4, 8, P, 2, 512).transpose(3, 0, 2, 1, 4)
    ).astype(np.float16)

    ones_host = np.ones((P, 130), np.float32)
    # additive causal mask for diagonal tiles: 0 where q >= k, -30000 where
    # q < k (exp(-30000/32) == 0)
    triu_host = ((np.triu(np.ones((P, P))) - 1.0) *
                 30000.0).astype(ml_dtypes.bfloat16)
    b_qkv_b = b_qkv.astype(ml_dtypes.bfloat16)
    b_proj_b = b_proj.astype(ml_dtypes.bfloat16)
    b1e_h = b1e.astype(np.float16)
    b2_h = b2.astype(np.float16)
    in_maps = []
    for c in range(NCORES):
        # xq [B, 3, 128, 8, 88]: pre-transposed x rows per slot:
        # [b, t, p, kc, r] = x[b, T0+r, kc*128+p]
        xq = np.zeros((B, 3, P, 8, 88), ml_dtypes.bfloat16)
        offs = np.zeros((1, 4), np.uint32)
        for t in range(3):
            start = (16 * t + 2 * c) * BLK
            T0 = start // ROW
            offs[0, t] = ROW - (start - T0 * ROW)
            n = min(88, S - T0)
            for b in range(B):
                xt = x[b, T0:T0 + n].T.reshape(8, P, n)   # [kc, p, r]
                xq[b, t, :, :, :n] = xt.transpose(1, 0, 2)
        xr = np.zeros((B, 2, P, E), np.float32)
        for hh in range(2):
            h_ = 2 * c + hh
            for b in range(B):
                xr[b, hh] = x[b, P * h_:P * (h_ + 1)]
        in_maps.append({
            "xq": xq, "xr": xr, "offs": offs,
            "ones": ones_host, "triu": triu_host,
            "wqkv": w_qkv_t, "bqkv": b_qkv_b, "wproj": w_proj_t,
            "bproj": b_proj_b,
            "w1e": w1e_t, "b1e": b1e_h, "w2": w2_t, "b2": b2_h,
        })
    return in_maps


def _run(inputs, trace=False, trace_cores=None):
    import sys
    if "/opt/trn_rl_repo" not in sys.path:
        sys.path.insert(0, "/opt/trn_rl_repo")
    from concourse.bass_utils import run_bass_kernel_spmd
    nc = _get_nc()
    in_maps = _make_in_maps(inputs)
    kwargs = {}
    if trace:
        kwargs["trace"] = True
        if trace_cores is not None:
            kwargs["trace_cores"] = trace_cores
    res = run_bass_kernel_spmd(nc, in_maps, list(range(NCORES)), **kwargs)
    full = np.zeros((B, S, E), np.float32)
    for c in range(NCORES):
        o = res.results[c]["out"]
        for hh in range(2):
            h_ = 2 * c + hh
            for b in range(B):
                full[b, P * h_:P * (h_ + 1)] = o[b, hh]
    return full, res


def kernel(**inputs) -> np.ndarray:
    import sys
    if "/opt/trn_rl_repo" not in sys.path:
        sys.path.insert(0, "/opt/trn_rl_repo")
    full, _ = _run(inputs)
    return full


# revision 47
# speedup vs baseline: 1.2856x; 1.2441x over previous
# Trainium2 Bass kernel for nn_MultiHeadTransformer (B=2, S=2048, E=1024, H=16, FF=4096).
#
# Sharding: 8-way tensor/head parallel with ZERO collectives. The reference's
# "faithful raw view" reshape (b, s, 3E) -> (b, 3, H, s, Dh) means q/k/v of head h
# are contiguous 512KB slices of the flat qkv output buffer. Each core computes
# the qkv rows covering exactly the 6 flat blocks (q/k/v x 2 heads) it owns, does
# attention for its 2 heads, and because the inverse raw view maps head h's output
# to token rows [128h, 128(h+1)), the proj/LN/FFN are row-local to the core.
# Per-core offsets within the row-aligned scratch differ mod 3072; they are
# supplied as a tiny uint32 input and applied with one dynamic-offset DMA per
# slot, keeping a single SPMD program.
#
# v2: weight-stationary loops (each weight loaded once, large DMAs), x slots
# pre-transposed on host, proj in bf16, softmax denominator via gpsimd
# partition-broadcast + DVE divide (no DRAM round trips), FFN relu on DVE.
import numpy as np

B, S, E, H, DH, FF = 2, 2048, 1024, 16, 64, 4096
ROW = 3 * E            # 3072 qkv columns
BLK = S * DH           # 131072 elements per (type, head) block
NCORES = 8
P = 128
INV_SCALE = 1.0 / float(np.sqrt(E))

_cached = {}


def _build(zero_bias=False):
    import concourse.bacc as bacc
    import concourse.bass as bass
    import concourse.mybir as mybir
    import concourse.tile as tile
    from concourse.masks import make_identity

    f32 = mybir.dt.float32
    bf16 = mybir.dt.bfloat16   # attention/QKV path
    fp16 = mybir.dt.float16    # FFN path (finer mantissa for weight casts)
    u32 = mybir.dt.uint32
    AF = mybir.ActivationFunctionType
    ALU = mybir.AluOpType

    nc = bacc.Bacc(trn_type="TRN2", target_bir_lowering=False, debug=False,
                   num_devices=NCORES)

    xq = nc.dram_tensor("xq", [B, 3, P, 8, 88], bf16, kind="ExternalInput").ap()
    xr = nc.dram_tensor("xr", [B, 2, P, E], f32, kind="ExternalInput").ap()
    wqkv = nc.dram_tensor("wqkv", [3, P, 8, 1024], bf16, kind="ExternalInput").ap()
    bqkv = nc.dram_tensor("bqkv", [1, ROW], bf16, kind="ExternalInput").ap()
    wproj = nc.dram_tensor("wproj", [2, P, 8, 512], bf16, kind="ExternalInput").ap()
    bproj = nc.dram_tensor("bproj", [1, E], bf16, kind="ExternalInput").ap()
    w1e = nc.dram_tensor("w1e", [16, P, 2, 8, P], fp16, kind="ExternalInput").ap()
    b1e = nc.dram_tensor("b1e", [1, FF], fp16, kind="ExternalInput").ap()
    w2 = nc.dram_tensor("w2", [2, 4, P, 8, 512], fp16, kind="ExternalInput").ap()
    b2 = nc.dram_tensor("b2", [1, E], fp16, kind="ExternalInput").ap()
    offs = nc.dram_tensor("offs", [1, 4], u32, kind="ExternalInput").ap()
    ones_in = nc.dram_tensor("ones", [P, 130], f32, kind="ExternalInput").ap()
    triu_in = nc.dram_tensor("triu", [P, P], bf16, kind="ExternalInput").ap()
    out = nc.dram_tensor("out", [B, 2, P, E], f32, kind="ExternalOutput").ap()

    mlist = [(b, hh) for b in range(B) for hh in range(2)]

    with tile.TileContext(nc) as tc:
        with tc.tile_pool(name="singles", bufs=1) as singles, \
             tc.tile_pool(name="dram", bufs=1, space="DRAM") as dram:

            ident = singles.tile([P, P], f32)
            make_identity(nc, ident)
            ident_bf = singles.tile([P, P], bf16)
            make_identity(nc, ident_bf)
            # triu_in[k, q] = 0 where q >= k (keep), -30000 where q < k: added
            # to diagonal score tiles pre-exp via a PE matmul so no
            # vector/gpsimd op sits between exp and the AV matmul.
            trin = singles.tile([P, P], bf16)
            nc.sync.dma_start(trin, triu_in)
            eps_t = singles.tile([P, 1], f32)
            nc.vector.memset(eps_t, 1e-5)
            bq_b = singles.tile([1, ROW], bf16)
            nc.sync.dma_start(bq_b, bqkv)
            bp_b = singles.tile([1, E], bf16)
            nc.sync.dma_start(bp_b, bproj)
            b1_b = singles.tile([1, FF], fp16)
            nc.sync.dma_start(b1_b, b1e)
            b2_b = singles.tile([1, E], fp16)
            nc.sync.dma_start(b2_b, b2)
            ones_col = singles.tile([P, 16], f32)
            nc.sync.dma_start(ones_col, ones_in[:, 0:16])
            ones_b = singles.tile([1, P], bf16)
            nc.vector.memset(ones_b, 1.0)
            ones_h = singles.tile([1, P], fp16)
            nc.vector.memset(ones_h, 1.0)
            ones_h512 = singles.tile([1, 512], fp16)
            nc.vector.memset(ones_h512, 1.0)
            # wproj stays resident (16KB/partition) so proj can run per-head,
            # interleaved into the attention phase.
            wp_sb = singles.tile([P, 2, 8, 512], bf16)
            for ns_i in range(2):
                nc.scalar.dma_start(wp_sb[:, ns_i], wproj[ns_i])
            offs_sb = singles.tile([1, 4], u32)
            nc.sync.dma_start(offs_sb, offs)
            off_v = [nc.values_load(offs_sb[:, t:t + 1], min_val=0, max_val=ROW,
                                    skip_runtime_bounds_check=True)
                     for t in range(3)]

            SCR88 = ROW + 88 * ROW
            scr = [[dram.tile([SCR88], bf16, tag=f"scr{b}{t}",
                              name=f"scr{b}_{t}") for t in range(3)]
                   for b in range(B)]

            def transpose_into(pool, dst, src_ap, tag="tp", idt=None, dt_=f32,
                               bufs=None):
                prows = src_ap.shape[0]
                pcols = src_ap.shape[1]
                idt = ident if idt is None else idt
                kw = {} if bufs is None else {"bufs": bufs}
                t_ps = pool.tile([P, P], dt_, tag=tag, name="t_ps", **kw)
                nc.tensor.transpose(t_ps[:pcols, :prows], src_ap,
                                    idt[:prows, :prows])
                nc.vector.tensor_copy(dst, t_ps[:pcols, :prows])

            # ---------------- Phase A: QKV ----------------
            # xT comes pre-transposed from the host; weight-stationary n6 loop
            # loads each wqkv column block exactly once.
            slots = [(b, t) for b in range(B) for t in range(3)]
            with tc.tile_pool(name="qkv_ps", bufs=4, space="PSUM") as qkv_ps, \
                 tc.tile_pool(name="qkv_sb", bufs=1) as qkv_sb, \
                 tc.tile_pool(name="wq_sb", bufs=2) as wq_sb:
                xT = qkv_sb.tile([P, 6, 8, 88], bf16)    # lhsT chunks per slot
                y_sb = qkv_sb.tile([88, 6, ROW], bf16)   # qkv rows per slot
                for m, (b, t) in enumerate(slots):
                    nc.sync.dma_start(xT[:, m], xq[b, t])
                # Weight chunk loaded in two kc-halves so MMs start earlier.
                for n2 in range(3):
                    wh = []
                    for h in range(2):
                        w_sb = wq_sb.tile([P, 4, 1024], bf16, tag=f"wq{h}",
                                          name=f"wq{n2}_{h}")
                        nc.scalar.dma_start(
                            w_sb, wqkv[n2][:, 4 * h:4 * h + 4])
                        wh.append(w_sb)
                    for grp in range(2):
                        ms = [3 * grp, 3 * grp + 1, 3 * grp + 2]
                        acc = [qkv_ps.tile([88, 1024], f32, tag="acc",
                                           name=f"qa{n2}_{m}") for m in ms]
                        if not zero_bias:
                            for i in range(3):
                                for nh in range(2):
                                    ns = slice(n2 * 1024 + nh * 512,
                                               n2 * 1024 + nh * 512 + 512)
                                    nc.tensor.matmul(
                                        acc[i][:, 512 * nh:512 * nh + 512],
                                        lhsT=ones_b[:, :88],
                                        rhs=bq_b[:, ns], start=True,
                                        stop=False)
                        for kc in range(8):
                            for i, m in enumerate(ms):
                                for nh in range(2):
                                    nc.tensor.matmul(
                                        acc[i][:, 512 * nh:512 * nh + 512],
                                        lhsT=xT[:, m, kc, :],
                                        rhs=wh[kc // 4][:, kc % 4,
                                                        512 * nh:512 * nh + 512],
                                        start=(zero_bias and kc == 0),
                                        stop=(kc == 7))
                        for i, m in enumerate(ms):
                            nc.vector.tensor_copy(
                                y_sb[:, m, n2 * 1024:(n2 + 1) * 1024], acc[i])
                for m, (b, t) in enumerate(slots):
                    dst = scr[b][t][bass.ds(off_v[t], 88 * ROW)]
                    nc.sync.dma_start(
                        dst.rearrange("(r c) -> r c", c=ROW), y_sb[:, m, :])

            # -------- Phases B+C (ln spans C..D) --------
            ln_pool_cm = tc.tile_pool(name="ln_pool", bufs=4)
            ln_pool = ln_pool_cm.__enter__()
            outT_cm = tc.tile_pool(name="outT_sb", bufs=1)
            outT_pool = outT_cm.__enter__()

            lnT = ln_pool.tile([P, 8, 4, P], fp16, tag="lnT", bufs=1)
            xr_sb = [ln_pool.tile([P, E], f32, tag="xr", bufs=4,
                                  name=f"xr{mi}") for mi in range(4)]
            for mi, (b, hh) in enumerate(mlist):
                nc.sync.dma_start(xr_sb[mi], xr[b, hh])

            # ---------------- Phase B: attention ----------------
            # q is processed in two 1024-wide halves so each outT accumulator
            # is 2 PSUM banks; bufs=2 lets two (head, half) pipelines overlap.
            with tc.tile_pool(name="at_ps", bufs=2, space="PSUM") as at_ps, \
                 tc.tile_pool(name="ot_ps", bufs=2, space="PSUM") as ot_ps, \
                 tc.tile_pool(name="at_sb", bufs=4) as at_sb, \
                 tc.tile_pool(name="qkv_in", bufs=3) as qkv_in, \
                 tc.tile_pool(name="head_sb", bufs=2) as head_sb:
                for mi, (b, hh) in enumerate(mlist):
                    base = ROW + hh * BLK
                    # qTb: transposed q duplicated in both partition halves;
                    # kTd: even k-blocks in partitions 0-63, odd in 64-127.
                    # The two score matmuls of a block pair then use different
                    # PE row-groups, so the second LDWEIGHTS pulls ahead
                    # during the first matmul.
                    qTb = head_sb.tile([P, S], bf16, tag="qT", name=f"qT{mi}")
                    kTd = head_sb.tile([P, S // 2], bf16, tag="kT",
                                       name=f"kT{mi}")
                    v_sb = head_sb.tile([P, 16, 65], bf16, tag="v",
                                        name=f"v{mi}")
                    nc.vector.tensor_copy(
                        v_sb[:, :, 64:65],
                        ones_col.rearrange("p (f o) -> p f o", o=1))
                    qn = qkv_in.tile([P, 16, DH], bf16, tag="qn",
                                     name=f"qn{mi}")
                    kn = qkv_in.tile([P, 16, DH], bf16, tag="kn",
                                     name=f"kn{mi}")
                    nc.sync.dma_start(
                        qn, scr[b][0][base:base + BLK]
                        .rearrange("(i p d) -> p i d", p=P, d=DH))
                    nc.sync.dma_start(
                        kn, scr[b][1][base:base + BLK]
                        .rearrange("(i p d) -> p i d", p=P, d=DH))
                    nc.sync.dma_start(
                        v_sb[:, :, 0:64], scr[b][2][base:base + BLK]
                        .rearrange("(i p d) -> p i d", p=P, d=DH))
                    for i in range(16):
                        t_ps = at_ps.tile([P, P], bf16, tag="sc", bufs=3,
                                          name="t_ps")
                        nc.tensor.transpose(t_ps[0:64, :], qn[:, i, :],
                                            ident_bf)
                        nc.tensor.transpose(t_ps[64:128, :], qn[:, i, :],
                                            ident_bf)
                        nc.vector.tensor_copy(qTb[:, i * P:(i + 1) * P], t_ps)
                    for p8 in range(8):
                        t_ps = at_ps.tile([P, P], bf16, tag="sc", bufs=3,
                                          name="t_ps")
                        nc.tensor.transpose(t_ps[0:64, :], kn[:, 2 * p8, :],
                                            ident_bf)
                        nc.tensor.transpose(t_ps[64:128, :],
                                            kn[:, 2 * p8 + 1, :], ident_bf)
                        nc.vector.tensor_copy(kTd[:, p8 * P:(p8 + 1) * P],
                                              t_ps)
                    oT_sb = outT_pool.tile([64, S], f32, tag="oTsb",
                                           bufs=2, name=f"oTsb{mi}")
                    LAG = 2
                    for gw in range(4):          # 512-wide q windows
                        Ws = 512 * gw
                        npairs = 2 * gw + 2
                        oTw = ot_ps.tile([65, 512], f32, tag="oT",
                                         name=f"oT{mi}_{gw}")
                        pend = []

                        def pop_av(half):
                            # one AV matmul (one half of a pending pair) so
                            # consecutive PE matmuls alternate PSUM banks
                            # (score bank vs oT bank) and fill/drain overlap.
                            if not pend:
                                return
                            p, a_sb, s0, s1 = pend[0]
                            if half == 0:
                                nc.tensor.matmul(
                                    oTw[:, s0:512], lhsT=v_sb[:, 2 * p, :],
                                    rhs=a_sb[:, s0:512],
                                    start=(p == 0), stop=False)
                            else:
                                nc.tensor.matmul(
                                    oTw[:, s1:512], lhsT=v_sb[:, 2 * p + 1, :],
                                    rhs=a_sb[:, 512 + s1:1024],
                                    start=False, stop=(p == npairs - 1))
                                pend.pop(0)

                        for p in range(npairs):
                            j0, j1 = 2 * p, 2 * p + 1
                            s0 = max(128 * j0 - Ws, 0)
                            s1 = max(128 * j1 - Ws, 0)
                            d0 = 128 * j0 >= Ws      # block straddles diagonal
                            d1 = 128 * j1 >= Ws
                            sc_ps = at_ps.tile([P, 1024], f32, tag="sc",
                                               bufs=3, name=f"sc{mi}_{gw}_{p}")
                            a_sb = at_sb.tile([P, 1024], bf16, tag="a",
                                              bufs=6, name=f"a{mi}_{gw}_{p}")
                            ready = len(pend) > LAG
                            nc.tensor.matmul(
                                sc_ps[:, s0:512],
                                lhsT=kTd[0:64, p * P:(p + 1) * P],
                                rhs=qTb[0:64, Ws + s0:Ws + 512],
                                start=True, stop=not d0)
                            if d0:
                                nc.tensor.matmul(
                                    sc_ps[:, s0:s0 + P], lhsT=ident_bf,
                                    rhs=trin, start=False, stop=True,
                                    skip_group_check=True)
                            if ready:
                                pop_av(0)
                            nc.tensor.matmul(
                                sc_ps[:, 512 + s1:1024],
                                lhsT=kTd[64:128, p * P:(p + 1) * P],
                                rhs=qTb[64:128, Ws + s1:Ws + 512],
                                start=True, stop=not d1)
                            if d1:
                                nc.tensor.matmul(
                                    sc_ps[:, 512 + s1:512 + s1 + P],
                                    lhsT=ident_bf, rhs=trin,
                                    start=False, stop=True,
                                    skip_group_check=True)
                            if ready:
                                pop_av(1)
                            nc.scalar.activation(a_sb, sc_ps, AF.Exp,
                                                 scale=float(INV_SCALE))
                            pend.append((p, a_sb, s0, s1))
                        while pend:
                            pop_av(0)
                            pop_av(1)
                        # normalize this window; only the partition-broadcast
                        # runs on gpsimd.
                        dn = at_sb.tile([1, 512], f32, tag="dn", bufs=4,
                                        name=f"dn{mi}_{gw}")
                        nc.vector.tensor_copy(dn, oTw[64:65, :])
                        nc.vector.reciprocal_approx_fast(dn, dn)
                        bc = at_sb.tile([64, 512], f32, tag="bc", bufs=4,
                                        name=f"bc{mi}_{gw}")
                        nc.gpsimd.partition_broadcast(bc, dn)
                        nc.vector.tensor_mul(oT_sb[:, Ws:Ws + 512],
                                             oTw[0:64, :], bc)
                    hT = head_sb.tile([P, 8, P], bf16, tag="hT",
                                      name=f"hT{mi}")
                    oT_r = oT_sb.rearrange("d (t a) -> d a t", a=16)
                    for kc in range(8):
                        for ah in range(2):
                            nc.vector.tensor_copy(
                                hT[64 * ah:64 * ah + 64, kc, :],
                                oT_r[:, 2 * kc + ah, :])
                    # ---- proj + residual + LN for this m-tile, interleaved
                    # so the PE gets a dense burst between heads and Phase C
                    # disappears from the critical path.
                    r_sb = at_sb.tile([P, E], f32, tag="r", bufs=2,
                                      name=f"r{mi}")
                    # b_proj is folded into xr on the host, so no bias matmul
                    pacc = [ot_ps.tile([P, 512], f32, tag="oT",
                                       name=f"pa{mi}_{ns_i}")
                            for ns_i in range(2)]
                    for kc in range(8):
                        for ns_i in range(2):
                            nc.tensor.matmul(pacc[ns_i], lhsT=hT[:, kc, :],
                                             rhs=wp_sb[:, ns_i, kc, :],
                                             start=(kc == 0), stop=(kc == 7))
                    for ns_i in range(2):
                        ns = slice(ns_i * 512, (ns_i + 1) * 512)
                        nc.vector.tensor_add(r_sb[:, ns], pacc[ns_i],
                                             xr_sb[mi][:, ns])
                    stats = at_sb.tile([P, 2, 6], f32, tag="stats", bufs=2,
                                       name=f"st{mi}")
                    for sg in range(2):
                        nc.vector.bn_stats(stats[:, sg, :],
                                           r_sb[:, sg * 512:(sg + 1) * 512])
                    mv = at_sb.tile([P, 2], f32, tag="mv", bufs=2,
                                    name=f"mv{mi}")
                    nc.vector.bn_aggr(mv, stats)
                    nc.scalar.activation(mv[:, 1:2], mv[:, 1:2], AF.Sqrt,
                                         bias=eps_t, scale=1.0)
                    nc.vector.reciprocal(mv[:, 1:2], mv[:, 1:2])
                    ln_m = at_sb.tile([P, E], f32, tag="ln", bufs=2,
                                      name=f"ln{mi}")
                    nc.vector.tensor_scalar(
                        ln_m, r_sb, mv[:, 0:1], mv[:, 1:2],
                        ALU.subtract, ALU.mult)
                    for kc in range(8):
                        transpose_into(at_ps, lnT[:, kc, mi, :],
                                       ln_m[:, kc * P:(kc + 1) * P],
                                       tag="sc", bufs=3)

            outT_cm.__exit__(None, None, None)

            # ---------------- Phase D: FFN ----------------
            # h1T computed directly: lhsT = w1 block (e,f), rhs = lnT over all
            # four m-tiles (e, 4*128 tokens) -> h1T (f, tokens). No h1
            # transposes needed, and h1T slices feed w2 as lhsT directly.
            with tc.tile_pool(name="ff_ps", bufs=4, space="PSUM") as ff_ps, \
                 tc.tile_pool(name="fo_ps", bufs=4, space="PSUM") as fo_ps, \
                 tc.tile_pool(name="ff_sb", bufs=1) as ff_sb, \
                 tc.tile_pool(name="wf_sb", bufs=3) as wf_sb, \
                 tc.tile_pool(name="w2_sb", bufs=2) as w2_sb, \
                 tc.tile_pool(name="o_sb", bufs=2) as o_pool:
                h1T = ff_sb.tile([P, 32, 4, P], fp16)   # (f-part, fc, token)
                for fcp in range(16):
                    w_sb = wf_sb.tile([P, 2, 8, P], fp16, tag="w1s",
                                      name=f"w1_{fcp}")
                    nc.scalar.dma_start(w_sb, w1e[fcp])
                    facc = [ff_ps.tile([P, 512], f32, tag="facc",
                                       name=f"fa{2 * fcp + fl}")
                            for fl in range(2)]
                    if not zero_bias:
                        for fl in range(2):
                            fc = 2 * fcp + fl
                            nc.tensor.matmul(
                                facc[fl],
                                lhsT=b1_b[:, 128 * fc:128 * (fc + 1)],
                                rhs=ones_h512, start=True, stop=False)
                    for kc in range(8):
                        for fl in range(2):
                            nc.tensor.matmul(
                                facc[fl], lhsT=w_sb[:, fl, kc, :],
                                rhs=lnT[:, kc, :, :],
                                start=(zero_bias and kc == 0),
                                stop=(kc == 7))
                    for fl in range(2):
                        nc.vector.tensor_relu(h1T[:, 2 * fcp + fl, :, :],
                                              facc[fl])
                for ns_i in range(2):
                    ns = slice(ns_i * 512, (ns_i + 1) * 512)
                    acc = [fo_ps.tile([P, 512], f32, tag="oacc",
                                      name=f"oa{ns_i}_{mi}") for mi in range(4)]
                    if not zero_bias:
                        for mi in range(4):
                            nc.tensor.matmul(acc[mi], lhsT=ones_h,
                                             rhs=b2_b[:, ns],
                                             start=True, stop=False)
                    for kb in range(4):
                        w_sb = w2_sb.tile([P, 8, 512], fp16, tag="w2s",
                                          name=f"w2_{ns_i}_{kb}")
                        nc.scalar.dma_start(w_sb, w2[ns_i, kb])
                        for kc8 in range(8):
                            kc = 8 * kb + kc8
                            for mi in range(4):
                                nc.tensor.matmul(acc[mi],
                                                 lhsT=h1T[:, kc, mi, :],
                                                 rhs=w_sb[:, kc8, :],
                                                 start=(zero_bias and kc == 0),
                                                 stop=(kc == 31))
                    for mi, (b, hh) in enumerate(mlist):
                        o_sb = o_pool.tile([P, 512], f32, tag="o",
                                           name=f"o{ns_i}_{mi}")
                        nc.vector.tensor_copy(o_sb, acc[mi])
                        nc.sync.dma_start(out[b, hh, :, ns], o_sb)

            ln_pool_cm.__exit__(None, None, None)

    nc.compile()
    return nc


def _get_nc(zero_bias=False):
    key = ("nc", zero_bias)
    if key not in _cached:
        _cached[key] = _build(zero_bias=zero_bias)
    return _cached[key]


def _make_in_maps(inputs):
    import ml_dtypes
    x = np.ascontiguousarray(np.asarray(inputs["x"], dtype=np.float32))
    w_qkv = np.ascontiguousarray(np.asarray(inputs["w_qkv"], dtype=np.float32))
    b_qkv = np.asarray(inputs["b_qkv"], dtype=np.float32).reshape(1, ROW)
    w_proj = np.ascontiguousarray(np.asarray(inputs["w_proj"], dtype=np.float32))
    b_proj = np.asarray(inputs["b_proj"], dtype=np.float32).reshape(1, E)
    ln_g = np.asarray(inputs["ln_g"], dtype=np.float32)
    ln_b = np.asarray(inputs["ln_b"], dtype=np.float32)
    w1 = np.asarray(inputs["w1"], dtype=np.float32)
    b1 = np.asarray(inputs["b1"], dtype=np.float32)
    w2 = np.ascontiguousarray(np.asarray(inputs["w2"], dtype=np.float32))
    b2 = np.asarray(inputs["b2"], dtype=np.float32).reshape(1, E)

    w1e = (ln_g[:, None] * w1).astype(np.float32)
    b1e = (b1 + ln_b @ w1).reshape(1, FF).astype(np.float32)

    # wqkv [3, 128, 8, 1024]: [n2, p, kc, f] = w_qkv[kc*128+p, n2*1024+f]
    w_qkv_t = np.ascontiguousarray(
        w_qkv.reshape(8, P, 3, 1024).transpose(2, 1, 0, 3)
    ).astype(ml_dtypes.bfloat16)
    # wproj [2, 128, 8, 512]
    w_proj_t = np.ascontiguousarray(
        w_proj.reshape(8, P, 2, 512).transpose(2, 1, 0, 3)
    ).astype(ml_dtypes.bfloat16)
    # w1e [16, 128, 2, 8, 128]: [fcp, p, fl, kc, f] = w1e[kc*128+p, (2*fcp+fl)*128+f]
    w1e_t = np.ascontiguousarray(
        w1e.reshape(8, P, 16, 2, P).transpose(2, 1, 3, 0, 4)
    ).astype(np.float16)
    # w2 [2, 4, 128, 8, 512]: [ns, kb, p, kc8, f] = w2[(kb*8+kc8)*128+p, ns*512+f]
    w2_t = np.ascontiguousarray(
        w2.reshape(4, 8, P, 2, 512).transpose(3, 0, 2, 1, 4)
    ).astype(np.float16)

    ones_host = np.ones((P, 130), np.float32)
    # additive causal mask for diagonal tiles: 0 where q >= k, -30000 where
    # q < k (exp(-30000/32) == 0)
    triu_host = ((np.triu(np.ones((P, P))) - 1.0) *
                 30000.0).astype(ml_dtypes.bfloat16)
    b_qkv_b = b_qkv.astype(ml_dtypes.bfloat16)
    b_proj_b = b_proj.astype(ml_dtypes.bfloat16)
    b1e_h = b1e.astype(np.float16)
    b2_h = b2.astype(np.float16)
    in_maps = []
    for c in range(NCORES):
        # xq [B, 3, 128, 8, 88]: pre-transposed x rows per slot:
        # [b, t, p, kc, r] = x[b, T0+r, kc*128+p]
        xq = np.zeros((B, 3, P, 8, 88), ml_dtypes.bfloat16)
        offs = np.zeros((1, 4), np.uint32)
        for t in range(3):
            start = (16 * t + 2 * c) * BLK
            T0 = start // ROW
            offs[0, t] = ROW - (start - T0 * ROW)
            n = min(88, S - T0)
            for b in range(B):
                xt = x[b, T0:T0 + n].T.reshape(8, P, n)   # [kc, p, r]
                xq[b, t, :, :, :n] = xt.transpose(1, 0, 2)
        # b_proj folded into the residual input
        xr = np.zeros((B, 2, P, E), np.float32)
        for hh in range(2):
            h_ = 2 * c + hh
            for b in range(B):
                xr[b, hh] = x[b, P * h_:P * (h_ + 1)] + b_proj
        in_maps.append({
            "xq": xq, "xr": xr, "offs": offs,
            "ones": ones_host, "triu": triu_host,
            "wqkv": w_qkv_t, "bqkv": b_qkv_b, "wproj": w_proj_t,
            "bproj": b_proj_b,
            "w1e": w1e_t, "b1e": b1e_h, "w2": w2_t, "b2": b2_h,
        })
    return in_maps


def _run(inputs, trace=False, trace_cores=None):
    import sys
    if "/opt/trn_rl_repo" not in sys.path:
        sys.path.insert(0, "/opt/trn_rl_repo")
    from concourse.bass_utils import run_bass_kernel_spmd
    zero_bias = bool(
        not np.any(np.asarray(inputs["b_qkv"]))
        and not np.any(np.asarray(inputs["b1"]))
        and not np.any(np.asarray(inputs["ln_b"]))
        and not np.any(np.asarray(inputs["b2"])))
    nc = _get_nc(zero_bias=zero_bias)
    in_maps = _make_in_maps(inputs)
    kwargs = {}
    if trace:
        kwargs["trace"] = True
        if trace_cores is not None:
            kwargs["trace_cores"] = trace_cores
    res = run_bass_kernel_spmd(nc, in_maps, list(range(NCORES)), **kwargs)
    full = np.zeros((B, S, E), np.float32)
    for c in range(NCORES):
        o = res.results[c]["out"]
        for hh in range(2):
            h_ = 2 * c + hh
            for b in range(B):
                full[b, P * h_:P * (h_ + 1)] = o[b, hh]
    return full, res


def kernel(**inputs) -> np.ndarray:
    import sys
    if "/opt/trn_rl_repo" not in sys.path:
        sys.path.insert(0, "/opt/trn_rl_repo")
    full, _ = _run(inputs)
    return full


# revision 54
# speedup vs baseline: 1.3231x; 1.0292x over previous
# Trainium2 Bass kernel for nn_MultiHeadTransformer (B=2, S=2048, E=1024, H=16, FF=4096).
#
# Sharding: 8-way tensor/head parallel with ZERO collectives. The reference's
# "faithful raw view" reshape (b, s, 3E) -> (b, 3, H, s, Dh) means q/k/v of head h
# are contiguous 512KB slices of the flat qkv output buffer. Each core computes
# the qkv rows covering exactly the 6 flat blocks (q/k/v x 2 heads) it owns, does
# attention for its 2 heads, and because the inverse raw view maps head h's output
# to token rows [128h, 128(h+1)), the proj/LN/FFN are row-local to the core.
# Per-core offsets within the row-aligned scratch differ mod 3072; they are
# supplied as a tiny uint32 input and applied with one dynamic-offset DMA per
# slot, keeping a single SPMD program.
#
# v2: weight-stationary loops (each weight loaded once, large DMAs), x slots
# pre-transposed on host, proj in bf16, softmax denominator via gpsimd
# partition-broadcast + DVE divide (no DRAM round trips), FFN relu on DVE.
import numpy as np

B, S, E, H, DH, FF = 2, 2048, 1024, 16, 64, 4096
ROW = 3 * E            # 3072 qkv columns
BLK = S * DH           # 131072 elements per (type, head) block
NCORES = 8
P = 128
INV_SCALE = 1.0 / float(np.sqrt(E))

_cached = {}


def _build(zero_bias=False):
    import concourse.bacc as bacc
    import concourse.bass as bass
    import concourse.mybir as mybir
    import concourse.tile as tile
    from concourse.masks import make_identity

    f32 = mybir.dt.float32
    bf16 = mybir.dt.bfloat16   # attention/QKV path
    fp16 = mybir.dt.float16    # FFN path (finer mantissa for weight casts)
    u32 = mybir.dt.uint32
    AF = mybir.ActivationFunctionType
    ALU = mybir.AluOpType

    nc = bacc.Bacc(trn_type="TRN2", target_bir_lowering=False, debug=False,
                   num_devices=NCORES)

    xq = nc.dram_tensor("xq", [B, 3, P, 8, 88], bf16, kind="ExternalInput").ap()
    xr = nc.dram_tensor("xr", [B, 2, P, E], f32, kind="ExternalInput").ap()
    wqkv = nc.dram_tensor("wqkv", [3, P, 8, 1024], bf16, kind="ExternalInput").ap()
    bqkv = nc.dram_tensor("bqkv", [1, ROW], bf16, kind="ExternalInput").ap()
    wproj = nc.dram_tensor("wproj", [2, P, 8, 512], bf16, kind="ExternalInput").ap()
    bproj = nc.dram_tensor("bproj", [1, E], bf16, kind="ExternalInput").ap()
    w1e = nc.dram_tensor("w1e", [16, P, 2, 8, P], fp16, kind="ExternalInput").ap()
    b1e = nc.dram_tensor("b1e", [1, FF], fp16, kind="ExternalInput").ap()
    w2 = nc.dram_tensor("w2", [2, 4, P, 8, 512], fp16, kind="ExternalInput").ap()
    b2 = nc.dram_tensor("b2", [1, E], fp16, kind="ExternalInput").ap()
    offs = nc.dram_tensor("offs", [1, 4], u32, kind="ExternalInput").ap()
    ones_in = nc.dram_tensor("ones", [P, 130], f32, kind="ExternalInput").ap()
    triu_in = nc.dram_tensor("triu", [P, P], bf16, kind="ExternalInput").ap()
    out = nc.dram_tensor("out", [B, 2, P, E], f32, kind="ExternalOutput").ap()

    mlist = [(b, hh) for b in range(B) for hh in range(2)]

    with tile.TileContext(nc) as tc:
        with tc.tile_pool(name="singles", bufs=1) as singles, \
             tc.tile_pool(name="dram", bufs=1, space="DRAM") as dram:

            ident = singles.tile([P, P], f32)
            make_identity(nc, ident)
            ident_bf = singles.tile([P, P], bf16)
            make_identity(nc, ident_bf)
            # triu_in[k, q] = 0 where q >= k (keep), -30000 where q < k: added
            # to diagonal score tiles pre-exp via a PE matmul so no
            # vector/gpsimd op sits between exp and the AV matmul.
            trin = singles.tile([P, P], bf16)
            nc.sync.dma_start(trin, triu_in)
            eps_t = singles.tile([P, 1], f32)
            nc.vector.memset(eps_t, 1e-5)
            bq_b = singles.tile([1, ROW], bf16)
            nc.sync.dma_start(bq_b, bqkv)
            bp_b = singles.tile([1, E], bf16)
            nc.sync.dma_start(bp_b, bproj)
            b1_b = singles.tile([1, FF], fp16)
            nc.sync.dma_start(b1_b, b1e)
            b2_b = singles.tile([1, E], fp16)
            nc.sync.dma_start(b2_b, b2)
            ones_col = singles.tile([P, 16], f32)
            nc.sync.dma_start(ones_col, ones_in[:, 0:16])
            ones_b = singles.tile([1, P], bf16)
            nc.vector.memset(ones_b, 1.0)
            ones_h = singles.tile([1, P], fp16)
            nc.vector.memset(ones_h, 1.0)
            ones_h512 = singles.tile([1, 512], fp16)
            nc.vector.memset(ones_h512, 1.0)
            # wproj stays resident (16KB/partition) so proj can run per-head,
            # interleaved into the attention phase.  Its DMA is issued after
            # Phase A's weight loads so it doesn't delay the first QKV matmul.
            wp_sb = singles.tile([P, 2, 8, 512], bf16)
            offs_sb = singles.tile([1, 4], u32)
            nc.sync.dma_start(offs_sb, offs)
            off_v = [nc.values_load(offs_sb[:, t:t + 1], min_val=0, max_val=ROW,
                                    skip_runtime_bounds_check=True)
                     for t in range(3)]

            SCR88 = ROW + 88 * ROW
            scr = [[dram.tile([SCR88], bf16, tag=f"scr{b}{t}",
                              name=f"scr{b}_{t}") for t in range(3)]
                   for b in range(B)]

            def transpose_into(pool, dst, src_ap, tag="tp", idt=None, dt_=f32,
                               bufs=None):
                prows = src_ap.shape[0]
                pcols = src_ap.shape[1]
                idt = ident if idt is None else idt
                kw = {} if bufs is None else {"bufs": bufs}
                t_ps = pool.tile([P, P], dt_, tag=tag, name="t_ps", **kw)
                nc.tensor.transpose(t_ps[:pcols, :prows], src_ap,
                                    idt[:prows, :prows])
                nc.vector.tensor_copy(dst, t_ps[:pcols, :prows])

            # ---------------- Phase A: QKV ----------------
            # xT comes pre-transposed from the host; weight-stationary n6 loop
            # loads each wqkv column block exactly once.
            slots = [(b, t) for b in range(B) for t in range(3)]
            with tc.tile_pool(name="qkv_ps", bufs=4, space="PSUM") as qkv_ps, \
                 tc.tile_pool(name="qkv_sb", bufs=1) as qkv_sb, \
                 tc.tile_pool(name="wq_sb", bufs=2) as wq_sb:
                xT = qkv_sb.tile([P, 6, 8, 88], bf16)    # lhsT chunks per slot
                y_sb = qkv_sb.tile([88, 6, ROW], bf16)   # qkv rows per slot
                for m, (b, t) in enumerate(slots):
                    nc.sync.dma_start(xT[:, m], xq[b, t])
                # Weight chunk loaded in two kc-halves so MMs start earlier.
                for n2 in range(3):
                    wh = []
                    for h in range(2):
                        w_sb = wq_sb.tile([P, 4, 1024], bf16, tag=f"wq{h}",
                                          name=f"wq{n2}_{h}")
                        nc.scalar.dma_start(
                            w_sb, wqkv[n2][:, 4 * h:4 * h + 4])
                        wh.append(w_sb)
                    for grp in range(2):
                        ms = [3 * grp, 3 * grp + 1, 3 * grp + 2]
                        acc = [qkv_ps.tile([88, 1024], f32, tag="acc",
                                           name=f"qa{n2}_{m}") for m in ms]
                        if not zero_bias:
                            for i in range(3):
                                for nh in range(2):
                                    ns = slice(n2 * 1024 + nh * 512,
                                               n2 * 1024 + nh * 512 + 512)
                                    nc.tensor.matmul(
                                        acc[i][:, 512 * nh:512 * nh + 512],
                                        lhsT=ones_b[:, :88],
                                        rhs=bq_b[:, ns], start=True,
                                        stop=False)
                        for kc in range(8):
                            for i, m in enumerate(ms):
                                for nh in range(2):
                                    nc.tensor.matmul(
                                        acc[i][:, 512 * nh:512 * nh + 512],
                                        lhsT=xT[:, m, kc, :],
                                        rhs=wh[kc // 4][:, kc % 4,
                                                        512 * nh:512 * nh + 512],
                                        start=(zero_bias and kc == 0),
                                        stop=(kc == 7))
                        for i, m in enumerate(ms):
                            nc.vector.tensor_copy(
                                y_sb[:, m, n2 * 1024:(n2 + 1) * 1024], acc[i])
                for m, (b, t) in enumerate(slots):
                    dst = scr[b][t][bass.ds(off_v[t], 88 * ROW)]
                    nc.sync.dma_start(
                        dst.rearrange("(r c) -> r c", c=ROW), y_sb[:, m, :])
                for ns_i in range(2):
                    nc.scalar.dma_start(wp_sb[:, ns_i], wproj[ns_i])

            # -------- Phases B+C (ln spans C..D) --------
            ln_pool_cm = tc.tile_pool(name="ln_pool", bufs=4)
            ln_pool = ln_pool_cm.__enter__()
            outT_cm = tc.tile_pool(name="outT_sb", bufs=1)
            outT_pool = outT_cm.__enter__()

            lnT = ln_pool.tile([P, 8, 4, P], fp16, tag="lnT", bufs=1)
            xr_sb = [ln_pool.tile([P, E], f32, tag="xr", bufs=4,
                                  name=f"xr{mi}") for mi in range(4)]
            for mi, (b, hh) in enumerate(mlist):
                nc.sync.dma_start(xr_sb[mi], xr[b, hh])

            # ---------------- Phase B: attention ----------------
            # q is processed in two 1024-wide halves so each outT accumulator
            # is 2 PSUM banks; bufs=2 lets two (head, half) pipelines overlap.
            with tc.tile_pool(name="at_ps", bufs=2, space="PSUM") as at_ps, \
                 tc.tile_pool(name="ot_ps", bufs=2, space="PSUM") as ot_ps, \
                 tc.tile_pool(name="at_sb", bufs=4) as at_sb, \
                 tc.tile_pool(name="qkv_in", bufs=3) as qkv_in, \
                 tc.tile_pool(name="head_sb", bufs=2) as head_sb:
                # proj + LN for head m are emitted one head later, so the LN
                # chain (DVE) overlaps the next head's attention instead of
                # gating the PE via in-order lnT transposes.
                pending_proj = None
                pending_ln = None

                def emit_proj_ln(ent):
                    pmi, phT = ent
                    r_sb = at_sb.tile([P, E], f32, tag="r", bufs=2,
                                      name=f"r{pmi}")
                    # b_proj is folded into xr on the host: no bias matmul
                    pacc = [ot_ps.tile([P, 512], f32, tag="oT",
                                       name=f"pa{pmi}_{ns_i}")
                            for ns_i in range(2)]
                    for kc in range(8):
                        for ns_i in range(2):
                            nc.tensor.matmul(pacc[ns_i], lhsT=phT[:, kc, :],
                                             rhs=wp_sb[:, ns_i, kc, :],
                                             start=(kc == 0), stop=(kc == 7))
                    for ns_i in range(2):
                        ns = slice(ns_i * 512, (ns_i + 1) * 512)
                        nc.vector.tensor_add(r_sb[:, ns], pacc[ns_i],
                                             xr_sb[pmi][:, ns])
                    stats = at_sb.tile([P, 2, 6], f32, tag="stats", bufs=2,
                                       name=f"st{pmi}")
                    for sg in range(2):
                        nc.vector.bn_stats(stats[:, sg, :],
                                           r_sb[:, sg * 512:(sg + 1) * 512])
                    mv = at_sb.tile([P, 2], f32, tag="mv", bufs=2,
                                    name=f"mv{pmi}")
                    nc.vector.bn_aggr(mv, stats)
                    nc.scalar.activation(mv[:, 1:2], mv[:, 1:2], AF.Sqrt,
                                         bias=eps_t, scale=1.0)
                    nc.vector.reciprocal(mv[:, 1:2], mv[:, 1:2])
                    ln_m = at_sb.tile([P, E], f32, tag="ln", bufs=2,
                                      name=f"ln{pmi}")
                    nc.vector.tensor_scalar(
                        ln_m, r_sb, mv[:, 0:1], mv[:, 1:2],
                        ALU.subtract, ALU.mult)
                    return (pmi, ln_m)

                def emit_lnT(ent):
                    pmi, ln_m = ent
                    for kc in range(8):
                        transpose_into(at_ps, lnT[:, kc, pmi, :],
                                       ln_m[:, kc * P:(kc + 1) * P],
                                       tag="sc", bufs=3)

                for mi, (b, hh) in enumerate(mlist):
                    base = ROW + hh * BLK
                    # qTb: transposed q duplicated in both partition halves;
                    # kTd: even k-blocks in partitions 0-63, odd in 64-127.
                    # The two score matmuls of a block pair then use different
                    # PE row-groups, so the second LDWEIGHTS pulls ahead
                    # during the first matmul.
                    qTb = head_sb.tile([P, S], bf16, tag="qT", name=f"qT{mi}")
                    kTd = head_sb.tile([P, S // 2], bf16, tag="kT",
                                       name=f"kT{mi}")
                    v_sb = head_sb.tile([P, 16, 65], bf16, tag="v",
                                        name=f"v{mi}")
                    nc.vector.tensor_copy(
                        v_sb[:, :, 64:65],
                        ones_col.rearrange("p (f o) -> p f o", o=1))
                    qn = qkv_in.tile([P, 16, DH], bf16, tag="qn",
                                     name=f"qn{mi}")
                    kn = qkv_in.tile([P, 16, DH], bf16, tag="kn",
                                     name=f"kn{mi}")
                    nc.sync.dma_start(
                        qn, scr[b][0][base:base + BLK]
                        .rearrange("(i p d) -> p i d", p=P, d=DH))
                    nc.sync.dma_start(
                        kn, scr[b][1][base:base + BLK]
                        .rearrange("(i p d) -> p i d", p=P, d=DH))
                    nc.sync.dma_start(
                        v_sb[:, :, 0:64], scr[b][2][base:base + BLK]
                        .rearrange("(i p d) -> p i d", p=P, d=DH))
                    for i in range(16):
                        t_ps = at_ps.tile([P, P], bf16, tag="sc", bufs=3,
                                          name="t_ps")
                        nc.tensor.transpose(t_ps[0:64, :], qn[:, i, :],
                                            ident_bf)
                        nc.tensor.transpose(t_ps[64:128, :], qn[:, i, :],
                                            ident_bf)
                        nc.vector.tensor_copy(qTb[:, i * P:(i + 1) * P], t_ps)
                    for p8 in range(8):
                        t_ps = at_ps.tile([P, P], bf16, tag="sc", bufs=3,
                                          name="t_ps")
                        nc.tensor.transpose(t_ps[0:64, :], kn[:, 2 * p8, :],
                                            ident_bf)
                        nc.tensor.transpose(t_ps[64:128, :],
                                            kn[:, 2 * p8 + 1, :], ident_bf)
                        nc.vector.tensor_copy(kTd[:, p8 * P:(p8 + 1) * P],
                                              t_ps)
                    if pending_proj is not None:
                        pending_ln = emit_proj_ln(pending_proj)
                        pending_proj = None
                    oT_sb = outT_pool.tile([64, S], f32, tag="oTsb",
                                           bufs=2, name=f"oTsb{mi}")
                    LAG = 2
                    for gw in range(4):          # 512-wide q windows
                        Ws = 512 * gw
                        npairs = 2 * gw + 2
                        oTw = ot_ps.tile([65, 512], f32, tag="oT",
                                         name=f"oT{mi}_{gw}")
                        pend = []

                        def pop_av(half):
                            # one AV matmul (one half of a pending pair) so
                            # consecutive PE matmuls alternate PSUM banks
                            # (score bank vs oT bank) and fill/drain overlap.
                            if not pend:
                                return
                            p, a_sb, s0, s1 = pend[0]
                            if half == 0:
                                nc.tensor.matmul(
                                    oTw[:, s0:512], lhsT=v_sb[:, 2 * p, :],
                                    rhs=a_sb[:, s0:512],
                                    start=(p == 0), stop=False)
                            else:
                                nc.tensor.matmul(
                                    oTw[:, s1:512], lhsT=v_sb[:, 2 * p + 1, :],
                                    rhs=a_sb[:, 512 + s1:1024],
                                    start=False, stop=(p == npairs - 1))
                                pend.pop(0)

                        for p in range(npairs):
                            j0, j1 = 2 * p, 2 * p + 1
                            s0 = max(128 * j0 - Ws, 0)
                            s1 = max(128 * j1 - Ws, 0)
                            d0 = 128 * j0 >= Ws      # block straddles diagonal
                            d1 = 128 * j1 >= Ws
                            sc_ps = at_ps.tile([P, 1024], f32, tag="sc",
                                               bufs=3, name=f"sc{mi}_{gw}_{p}")
                            a_sb = at_sb.tile([P, 1024], bf16, tag="a",
                                              bufs=6, name=f"a{mi}_{gw}_{p}")
                            ready = len(pend) > LAG
                            nc.tensor.matmul(
                                sc_ps[:, s0:512],
                                lhsT=kTd[0:64, p * P:(p + 1) * P],
                                rhs=qTb[0:64, Ws + s0:Ws + 512],
                                start=True, stop=not d0)
                            if d0:
                                nc.tensor.matmul(
                                    sc_ps[:, s0:s0 + P], lhsT=ident_bf,
                                    rhs=trin, start=False, stop=True,
                                    skip_group_check=True)
                            if ready:
                                pop_av(0)
                            nc.tensor.matmul(
                                sc_ps[:, 512 + s1:1024],
                                lhsT=kTd[64:128, p * P:(p + 1) * P],
                                rhs=qTb[64:128, Ws + s1:Ws + 512],
                                start=True, stop=not d1)
                            if d1:
                                nc.tensor.matmul(
                                    sc_ps[:, 512 + s1:512 + s1 + P],
                                    lhsT=ident_bf, rhs=trin,
                                    start=False, stop=True,
                                    skip_group_check=True)
                            if ready:
                                pop_av(1)
                            nc.scalar.activation(a_sb, sc_ps, AF.Exp,
                                                 scale=float(INV_SCALE))
                            pend.append((p, a_sb, s0, s1))
                        while pend:
                            pop_av(0)
                            pop_av(1)
                        # normalize this window; only the partition-broadcast
                        # runs on gpsimd.
                        dn = at_sb.tile([1, 512], f32, tag="dn", bufs=4,
                                        name=f"dn{mi}_{gw}")
                        nc.vector.tensor_copy(dn, oTw[64:65, :])
                        nc.vector.reciprocal_approx_fast(dn, dn)
                        bc = at_sb.tile([64, 512], f32, tag="bc", bufs=4,
                                        name=f"bc{mi}_{gw}")
                        nc.gpsimd.partition_broadcast(bc, dn)
                        nc.vector.tensor_mul(oT_sb[:, Ws:Ws + 512],
                                             oTw[0:64, :], bc)
                        if gw == 1 and pending_ln is not None:
                            emit_lnT(pending_ln)
                            pending_ln = None
                    hT = head_sb.tile([P, 8, P], bf16, tag="hT",
                                      name=f"hT{mi}")
                    oT_r = oT_sb.rearrange("d (t a) -> d a t", a=16)
                    for kc in range(8):
                        for ah in range(2):
                            nc.gpsimd.tensor_copy(
                                hT[64 * ah:64 * ah + 64, kc, :],
                                oT_r[:, 2 * kc + ah, :])
                    pending_proj = (mi, hT)
                emit_lnT(emit_proj_ln(pending_proj))

            outT_cm.__exit__(None, None, None)

            # ---------------- Phase D: FFN ----------------
            # h1T computed directly: lhsT = w1 block (e,f), rhs = lnT over all
            # four m-tiles (e, 4*128 tokens) -> h1T (f, tokens). No h1
            # transposes needed, and h1T slices feed w2 as lhsT directly.
            with tc.tile_pool(name="ff_ps", bufs=4, space="PSUM") as ff_ps, \
                 tc.tile_pool(name="fo_ps", bufs=4, space="PSUM") as fo_ps, \
                 tc.tile_pool(name="ff_sb", bufs=1) as ff_sb, \
                 tc.tile_pool(name="wf_sb", bufs=3) as wf_sb, \
                 tc.tile_pool(name="w2_sb", bufs=2) as w2_sb, \
                 tc.tile_pool(name="o_sb", bufs=2) as o_pool:
                h1T = ff_sb.tile([P, 32, 4, P], fp16)   # (f-part, fc, token)
                for fcp in range(16):
                    w_sb = wf_sb.tile([P, 2, 8, P], fp16, tag="w1s",
                                      name=f"w1_{fcp}")
                    nc.scalar.dma_start(w_sb, w1e[fcp])
                    facc = [ff_ps.tile([P, 512], f32, tag="facc",
                                       name=f"fa{2 * fcp + fl}")
                            for fl in range(2)]
                    if not zero_bias:
                        for fl in range(2):
                            fc = 2 * fcp + fl
                            nc.tensor.matmul(
                                facc[fl],
                                lhsT=b1_b[:, 128 * fc:128 * (fc + 1)],
                                rhs=ones_h512, start=True, stop=False)
                    for kc in range(8):
                        for fl in range(2):
                            nc.tensor.matmul(
                                facc[fl], lhsT=w_sb[:, fl, kc, :],
                                rhs=lnT[:, kc, :, :],
                                start=(zero_bias and kc == 0),
                                stop=(kc == 7))
                    for fl in range(2):
                        nc.vector.tensor_relu(h1T[:, 2 * fcp + fl, :, :],
                                              facc[fl])
                for ns_i in range(2):
                    ns = slice(ns_i * 512, (ns_i + 1) * 512)
                    acc = [fo_ps.tile([P, 512], f32, tag="oacc",
                                      name=f"oa{ns_i}_{mi}") for mi in range(4)]
                    if not zero_bias:
                        for mi in range(4):
                            nc.tensor.matmul(acc[mi], lhsT=ones_h,
                                             rhs=b2_b[:, ns],
                                             start=True, stop=False)
                    for kb in range(4):
                        w_sb = w2_sb.tile([P, 8, 512], fp16, tag="w2s",
                                          name=f"w2_{ns_i}_{kb}")
                        nc.scalar.dma_start(w_sb, w2[ns_i, kb])
                        for kc8 in range(8):
                            kc = 8 * kb + kc8
                            for mi in range(4):
                                nc.tensor.matmul(acc[mi],
                                                 lhsT=h1T[:, kc, mi, :],
                                                 rhs=w_sb[:, kc8, :],
                                                 start=(zero_bias and kc == 0),
                                                 stop=(kc == 31))
                    for mi, (b, hh) in enumerate(mlist):
                        o_sb = o_pool.tile([P, 512], f32, tag="o",
                                           name=f"o{ns_i}_{mi}")
                        nc.vector.tensor_copy(o_sb, acc[mi])
                        nc.sync.dma_start(out[b, hh, :, ns], o_sb)

            ln_pool_cm.__exit__(None, None, None)

    nc.compile()
    return nc


def _get_nc(zero_bias=False):
    key = ("nc", zero_bias)
    if key not in _cached:
        _cached[key] = _build(zero_bias=zero_bias)
    return _cached[key]


def _make_in_maps(inputs):
    import ml_dtypes
    x = np.ascontiguousarray(np.asarray(inputs["x"], dtype=np.float32))
    w_qkv = np.ascontiguousarray(np.asarray(inputs["w_qkv"], dtype=np.float32))
    b_qkv = np.asarray(inputs["b_qkv"], dtype=np.float32).reshape(1, ROW)
    w_proj = np.ascontiguousarray(np.asarray(inputs["w_proj"], dtype=np.float32))
    b_proj = np.asarray(inputs["b_proj"], dtype=np.float32).reshape(1, E)
    ln_g = np.asarray(inputs["ln_g"], dtype=np.float32)
    ln_b = np.asarray(inputs["ln_b"], dtype=np.float32)
    w1 = np.asarray(inputs["w1"], dtype=np.float32)
    b1 = np.asarray(inputs["b1"], dtype=np.float32)
    w2 = np.ascontiguousarray(np.asarray(inputs["w2"], dtype=np.float32))
    b2 = np.asarray(inputs["b2"], dtype=np.float32).reshape(1, E)

    w1e = (ln_g[:, None] * w1).astype(np.float32)
    b1e = (b1 + ln_b @ w1).reshape(1, FF).astype(np.float32)

    # wqkv [3, 128, 8, 1024]: [n2, p, kc, f] = w_qkv[kc*128+p, n2*1024+f]
    w_qkv_t = np.ascontiguousarray(
        w_qkv.reshape(8, P, 3, 1024).transpose(2, 1, 0, 3)
    ).astype(ml_dtypes.bfloat16)
    # wproj [2, 128, 8, 512]
    w_proj_t = np.ascontiguousarray(
        w_proj.reshape(8, P, 2, 512).transpose(2, 1, 0, 3)
    ).astype(ml_dtypes.bfloat16)
    # w1e [16, 128, 2, 8, 128]: [fcp, p, fl, kc, f] = w1e[kc*128+p, (2*fcp+fl)*128+f]
    w1e_t = np.ascontiguousarray(
        w1e.reshape(8, P, 16, 2, P).transpose(2, 1, 3, 0, 4)
    ).astype(np.float16)
    # w2 [2, 4, 128, 8, 512]: [ns, kb, p, kc8, f] = w2[(kb*8+kc8)*128+p, ns*512+f]
    w2_t = np.ascontiguousarray(
        w2.reshape(4, 8, P, 2, 512).transpose(3, 0, 2, 1, 4)
    ).astype(np.float16)

    ones_host = np.ones((P, 130), np.float32)
    # additive causal mask for diagonal tiles: 0 where q >= k, -30000 where
    # q < k (exp(-30000/32) == 0)
    triu_host = ((np.triu(np.ones((P, P))) - 1.0) *
                 30000.0).astype(ml_dtypes.bfloat16)
    b_qkv_b = b_qkv.astype(ml_dtypes.bfloat16)
    b_proj_b = b_proj.astype(ml_dtypes.bfloat16)
    b1e_h = b1e.astype(np.float16)
    b2_h = b2.astype(np.float16)
    in_maps = []
    for c in range(NCORES):
        # xq [B, 3, 128, 8, 88]: pre-transposed x rows per slot:
        # [b, t, p, kc, r] = x[b, T0+r, kc*128+p]
        xq = np.zeros((B, 3, P, 8, 88), ml_dtypes.bfloat16)
        offs = np.zeros((1, 4), np.uint32)
        for t in range(3):
            start = (16 * t + 2 * c) * BLK
            T0 = start // ROW
            offs[0, t] = ROW - (start - T0 * ROW)
            n = min(88, S - T0)
            for b in range(B):
                xt = x[b, T0:T0 + n].T.reshape(8, P, n)   # [kc, p, r]
                xq[b, t, :, :, :n] = xt.transpose(1, 0, 2)
        # b_proj folded into the residual input
        xr = np.zeros((B, 2, P, E), np.float32)
        for hh in range(2):
            h_ = 2 * c + hh
            for b in range(B):
                xr[b, hh] = x[b, P * h_:P * (h_ + 1)] + b_proj
        in_maps.append({
            "xq": xq, "xr": xr, "offs": offs,
            "ones": ones_host, "triu": triu_host,
            "wqkv": w_qkv_t, "bqkv": b_qkv_b, "wproj": w_proj_t,
            "bproj": b_proj_b,
            "w1e": w1e_t, "b1e": b1e_h, "w2": w2_t, "b2": b2_h,
        })
    return in_maps


def _run(inputs, trace=False, trace_cores=None):
    import sys
    if "/opt/trn_rl_repo" not in sys.path:
        sys.path.insert(0, "/opt/trn_rl_repo")
    from concourse.bass_utils import run_bass_kernel_spmd
    zero_bias = bool(
        not np.any(np.asarray(inputs["b_qkv"]))
        and not np.any(np.asarray(inputs["b1"]))
        and not np.any(np.asarray(inputs["ln_b"]))
        and not np.any(np.asarray(inputs["b2"])))
    nc = _get_nc(zero_bias=zero_bias)
    in_maps = _make_in_maps(inputs)
    kwargs = {}
    if trace:
        kwargs["trace"] = True
        if trace_cores is not None:
            kwargs["trace_cores"] = trace_cores
    res = run_bass_kernel_spmd(nc, in_maps, list(range(NCORES)), **kwargs)
    full = np.zeros((B, S, E), np.float32)
    for c in range(NCORES):
        o = res.results[c]["out"]
        for hh in range(2):
            h_ = 2 * c + hh
            for b in range(B):
                full[b, P * h_:P * (h_ + 1)] = o[b, hh]
    return full, res


def kernel(**inputs) -> np.ndarray:
    import sys
    if "/opt/trn_rl_repo" not in sys.path:
        sys.path.insert(0, "/opt/trn_rl_repo")
    full, _ = _run(inputs)
    return full


# revision 60
# speedup vs baseline: 1.3630x; 1.0301x over previous
# Trainium2 Bass kernel for nn_MultiHeadTransformer (B=2, S=2048, E=1024, H=16, FF=4096).
#
# Sharding: 8-way tensor/head parallel with ZERO collectives. The reference's
# "faithful raw view" reshape (b, s, 3E) -> (b, 3, H, s, Dh) means q/k/v of head h
# are contiguous 512KB slices of the flat qkv output buffer. Each core computes
# the qkv rows covering exactly the 6 flat blocks (q/k/v x 2 heads) it owns, does
# attention for its 2 heads, and because the inverse raw view maps head h's output
# to token rows [128h, 128(h+1)), the proj/LN/FFN are row-local to the core.
# Per-core offsets within the row-aligned scratch differ mod 3072; they are
# supplied as a tiny uint32 input and applied with one dynamic-offset DMA per
# slot, keeping a single SPMD program.
#
# v2: weight-stationary loops (each weight loaded once, large DMAs), x slots
# pre-transposed on host, proj in bf16, softmax denominator via gpsimd
# partition-broadcast + DVE divide (no DRAM round trips), FFN relu on DVE.
import numpy as np

B, S, E, H, DH, FF = 2, 2048, 1024, 16, 64, 4096
ROW = 3 * E            # 3072 qkv columns
BLK = S * DH           # 131072 elements per (type, head) block
NCORES = 8
P = 128
INV_SCALE = 1.0 / float(np.sqrt(E))

_cached = {}


def _build(zero_bias=False):
    import concourse.bacc as bacc
    import concourse.bass as bass
    import concourse.mybir as mybir
    import concourse.tile as tile
    from concourse.masks import make_identity

    f32 = mybir.dt.float32
    bf16 = mybir.dt.bfloat16   # attention/QKV path
    fp16 = mybir.dt.float16    # FFN path (finer mantissa for weight casts)
    u32 = mybir.dt.uint32
    AF = mybir.ActivationFunctionType
    ALU = mybir.AluOpType

    nc = bacc.Bacc(trn_type="TRN2", target_bir_lowering=False, debug=False,
                   num_devices=NCORES)

    xq = nc.dram_tensor("xq", [B, 3, P, 8, 88], bf16, kind="ExternalInput").ap()
    xr = nc.dram_tensor("xr", [B, 2, P, E], f32, kind="ExternalInput").ap()
    wqkv = nc.dram_tensor("wqkv", [3, P, 8, 1024], bf16, kind="ExternalInput").ap()
    bqkv = nc.dram_tensor("bqkv", [1, ROW], bf16, kind="ExternalInput").ap()
    wproj = nc.dram_tensor("wproj", [2, P, 8, 512], bf16, kind="ExternalInput").ap()
    bproj = nc.dram_tensor("bproj", [1, E], bf16, kind="ExternalInput").ap()
    w1e = nc.dram_tensor("w1e", [16, P, 2, 8, P], fp16, kind="ExternalInput").ap()
    b1e = nc.dram_tensor("b1e", [1, FF], fp16, kind="ExternalInput").ap()
    w2 = nc.dram_tensor("w2", [2, 4, P, 8, 512], fp16, kind="ExternalInput").ap()
    b2 = nc.dram_tensor("b2", [1, E], fp16, kind="ExternalInput").ap()
    offs = nc.dram_tensor("offs", [1, 4], u32, kind="ExternalInput").ap()
    ones_in = nc.dram_tensor("ones", [P, 130], f32, kind="ExternalInput").ap()
    triu_in = nc.dram_tensor("triu", [P, P], bf16, kind="ExternalInput").ap()
    out = nc.dram_tensor("out", [B, 2, P, E], f32, kind="ExternalOutput").ap()

    mlist = [(b, hh) for b in range(B) for hh in range(2)]

    with tile.TileContext(nc) as tc:
        with tc.tile_pool(name="singles", bufs=1) as singles, \
             tc.tile_pool(name="dram", bufs=1, space="DRAM") as dram:

            ident = singles.tile([P, P], f32)
            make_identity(nc, ident)
            ident_bf = singles.tile([P, P], bf16)
            make_identity(nc, ident_bf)
            # triu_in[k, q] = 0 where q >= k (keep), -30000 where q < k: added
            # to diagonal score tiles pre-exp via a PE matmul so no
            # vector/gpsimd op sits between exp and the AV matmul.
            trin = singles.tile([P, P], bf16)
            nc.sync.dma_start(trin, triu_in)
            eps_t = singles.tile([P, 1], f32)
            nc.vector.memset(eps_t, 1e-5)
            bq_b = singles.tile([1, ROW], bf16)
            nc.sync.dma_start(bq_b, bqkv)
            bp_b = singles.tile([1, E], bf16)
            nc.sync.dma_start(bp_b, bproj)
            b1_b = singles.tile([1, FF], fp16)
            nc.sync.dma_start(b1_b, b1e)
            b2_b = singles.tile([1, E], fp16)
            nc.sync.dma_start(b2_b, b2)
            ones_col = singles.tile([P, 16], f32)
            nc.sync.dma_start(ones_col, ones_in[:, 0:16])
            ones_b = singles.tile([1, P], bf16)
            nc.vector.memset(ones_b, 1.0)
            ones_h = singles.tile([1, P], fp16)
            nc.vector.memset(ones_h, 1.0)
            ones_h512 = singles.tile([1, 512], fp16)
            nc.vector.memset(ones_h512, 1.0)
            # wproj stays resident (16KB/partition) so proj can run per-head,
            # interleaved into the attention phase.  Its DMA is issued after
            # Phase A's weight loads so it doesn't delay the first QKV matmul.
            wp_sb = singles.tile([P, 2, 8, 512], bf16)
            offs_sb = singles.tile([1, 4], u32)
            nc.sync.dma_start(offs_sb, offs)
            off_v = [nc.values_load(offs_sb[:, t:t + 1], min_val=0, max_val=ROW,
                                    skip_runtime_bounds_check=True)
                     for t in range(3)]

            SCR88 = ROW + 88 * ROW
            scr = [[dram.tile([SCR88], bf16, tag=f"scr{b}{t}",
                              name=f"scr{b}_{t}") for t in range(3)]
                   for b in range(B)]

            def transpose_into(pool, dst, src_ap, tag="tp", idt=None, dt_=f32,
                               bufs=None):
                prows = src_ap.shape[0]
                pcols = src_ap.shape[1]
                idt = ident if idt is None else idt
                kw = {} if bufs is None else {"bufs": bufs}
                t_ps = pool.tile([P, P], dt_, tag=tag, name="t_ps", **kw)
                nc.tensor.transpose(t_ps[:pcols, :prows], src_ap,
                                    idt[:prows, :prows])
                nc.vector.tensor_copy(dst, t_ps[:pcols, :prows])

            # ---------------- Phase A: QKV ----------------
            # xT comes pre-transposed from the host; weight-stationary n6 loop
            # loads each wqkv column block exactly once.
            slots = [(b, t) for b in range(B) for t in range(3)]
            with tc.tile_pool(name="qkv_ps", bufs=4, space="PSUM") as qkv_ps, \
                 tc.tile_pool(name="qkv_sb", bufs=1) as qkv_sb, \
                 tc.tile_pool(name="wq_sb", bufs=2) as wq_sb:
                xT = qkv_sb.tile([P, 6, 8, 88], bf16)    # lhsT chunks per slot
                y_sb = qkv_sb.tile([88, 6, ROW], bf16)   # qkv rows per slot
                for m, (b, t) in enumerate(slots):
                    nc.sync.dma_start(xT[:, m], xq[b, t])
                # Weight chunk loaded in two kc-halves so MMs start earlier.
                for n2 in range(3):
                    wh = []
                    for h in range(4):
                        w_sb = wq_sb.tile([P, 2, 1024], bf16, tag=f"wq{h}",
                                          name=f"wq{n2}_{h}")
                        nc.scalar.dma_start(
                            w_sb, wqkv[n2][:, 2 * h:2 * h + 2])
                        wh.append(w_sb)
                    for grp in range(2):
                        ms = [3 * grp, 3 * grp + 1, 3 * grp + 2]
                        acc = [qkv_ps.tile([88, 1024], f32, tag="acc",
                                           name=f"qa{n2}_{m}") for m in ms]
                        if not zero_bias:
                            for i in range(3):
                                for nh in range(2):
                                    ns = slice(n2 * 1024 + nh * 512,
                                               n2 * 1024 + nh * 512 + 512)
                                    nc.tensor.matmul(
                                        acc[i][:, 512 * nh:512 * nh + 512],
                                        lhsT=ones_b[:, :88],
                                        rhs=bq_b[:, ns], start=True,
                                        stop=False)
                        for kc in range(8):
                            for i, m in enumerate(ms):
                                for nh in range(2):
                                    nc.tensor.matmul(
                                        acc[i][:, 512 * nh:512 * nh + 512],
                                        lhsT=xT[:, m, kc, :],
                                        rhs=wh[kc // 2][:, kc % 2,
                                                        512 * nh:512 * nh + 512],
                                        start=(zero_bias and kc == 0),
                                        stop=(kc == 7))
                        for i, m in enumerate(ms):
                            nc.vector.tensor_copy(
                                y_sb[:, m, n2 * 1024:(n2 + 1) * 1024], acc[i])
                for m, (b, t) in enumerate(slots):
                    dst = scr[b][t][bass.ds(off_v[t], 88 * ROW)]
                    nc.sync.dma_start(
                        dst.rearrange("(r c) -> r c", c=ROW), y_sb[:, m, :])
                for ns_i in range(2):
                    nc.scalar.dma_start(wp_sb[:, ns_i], wproj[ns_i])

            # -------- Phases B+C (ln spans C..D) --------
            ln_pool_cm = tc.tile_pool(name="ln_pool", bufs=4)
            ln_pool = ln_pool_cm.__enter__()
            outT_cm = tc.tile_pool(name="outT_sb", bufs=1)
            outT_pool = outT_cm.__enter__()

            lnT = ln_pool.tile([P, 8, 4, P], fp16, tag="lnT", bufs=1)
            xr_sb = [ln_pool.tile([P, E], f32, tag="xr", bufs=4,
                                  name=f"xr{mi}") for mi in range(4)]
            for mi, (b, hh) in enumerate(mlist):
                nc.sync.dma_start(xr_sb[mi], xr[b, hh])

            # ---------------- Phase B: attention ----------------
            # q is processed in two 1024-wide halves so each outT accumulator
            # is 2 PSUM banks; bufs=2 lets two (head, half) pipelines overlap.
            with tc.tile_pool(name="at_ps", bufs=2, space="PSUM") as at_ps, \
                 tc.tile_pool(name="ot_ps", bufs=2, space="PSUM") as ot_ps, \
                 tc.tile_pool(name="at_sb", bufs=4) as at_sb, \
                 tc.tile_pool(name="qkv_in", bufs=3) as qkv_in, \
                 tc.tile_pool(name="head_sb", bufs=2) as head_sb:
                # proj + LN for head m are emitted one head later, so the LN
                # chain (DVE) overlaps the next head's attention instead of
                # gating the PE via in-order lnT transposes.
                pending_proj = None
                pending_ln = None

                def emit_proj_ln(ent):
                    pmi, phT = ent
                    r_sb = at_sb.tile([P, E], f32, tag="r", bufs=2,
                                      name=f"r{pmi}")
                    # b_proj is folded into xr on the host: no bias matmul
                    pacc = [ot_ps.tile([P, 512], f32, tag="oT",
                                       name=f"pa{pmi}_{ns_i}")
                            for ns_i in range(2)]
                    for kc in range(8):
                        for ns_i in range(2):
                            nc.tensor.matmul(pacc[ns_i], lhsT=phT[:, kc, :],
                                             rhs=wp_sb[:, ns_i, kc, :],
                                             start=(kc == 0), stop=(kc == 7))
                    for ns_i in range(2):
                        ns = slice(ns_i * 512, (ns_i + 1) * 512)
                        nc.vector.tensor_add(r_sb[:, ns], pacc[ns_i],
                                             xr_sb[pmi][:, ns])
                    stats = at_sb.tile([P, 2, 6], f32, tag="stats", bufs=2,
                                       name=f"st{pmi}")
                    for sg in range(2):
                        nc.vector.bn_stats(stats[:, sg, :],
                                           r_sb[:, sg * 512:(sg + 1) * 512])
                    mv = at_sb.tile([P, 2], f32, tag="mv", bufs=2,
                                    name=f"mv{pmi}")
                    nc.vector.bn_aggr(mv, stats)
                    nc.scalar.activation(mv[:, 1:2], mv[:, 1:2], AF.Sqrt,
                                         bias=eps_t, scale=1.0)
                    nc.vector.reciprocal(mv[:, 1:2], mv[:, 1:2])
                    ln_m = at_sb.tile([P, E], f32, tag="ln", bufs=2,
                                      name=f"ln{pmi}")
                    nc.vector.tensor_scalar(
                        ln_m, r_sb, mv[:, 0:1], mv[:, 1:2],
                        ALU.subtract, ALU.mult)
                    return (pmi, ln_m)

                def emit_lnT(ent):
                    pmi, ln_m = ent
                    for kc in range(8):
                        transpose_into(at_ps, lnT[:, kc, pmi, :],
                                       ln_m[:, kc * P:(kc + 1) * P],
                                       tag="sc", bufs=3)

                for mi, (b, hh) in enumerate(mlist):
                    base = ROW + hh * BLK
                    # qTb: transposed q duplicated in both partition halves;
                    # kTd: even k-blocks in partitions 0-63, odd in 64-127.
                    # The two score matmuls of a block pair then use different
                    # PE row-groups, so the second LDWEIGHTS pulls ahead
                    # during the first matmul.
                    qTb = head_sb.tile([P, S], bf16, tag="qT", name=f"qT{mi}")
                    kTd = head_sb.tile([P, S // 2], bf16, tag="kT",
                                       name=f"kT{mi}")
                    v_sb = head_sb.tile([P, 16, 65], bf16, tag="v",
                                        name=f"v{mi}")
                    nc.vector.tensor_copy(
                        v_sb[:, :, 64:65],
                        ones_col.rearrange("p (f o) -> p f o", o=1))
                    qn = qkv_in.tile([P, 16, DH], bf16, tag="qn",
                                     name=f"qn{mi}")
                    kn = qkv_in.tile([P, 16, DH], bf16, tag="kn",
                                     name=f"kn{mi}")
                    nc.sync.dma_start(
                        qn, scr[b][0][base:base + BLK]
                        .rearrange("(i p d) -> p i d", p=P, d=DH))
                    nc.sync.dma_start(
                        kn, scr[b][1][base:base + BLK]
                        .rearrange("(i p d) -> p i d", p=P, d=DH))
                    nc.sync.dma_start(
                        v_sb[:, :, 0:64], scr[b][2][base:base + BLK]
                        .rearrange("(i p d) -> p i d", p=P, d=DH))
                    for i in range(16):
                        t_ps = at_ps.tile([P, P], bf16, tag="sc", bufs=3,
                                          name="t_ps")
                        nc.tensor.transpose(t_ps[0:64, :], qn[:, i, :],
                                            ident_bf)
                        nc.tensor.transpose(t_ps[64:128, :], qn[:, i, :],
                                            ident_bf)
                        nc.vector.tensor_copy(qTb[:, i * P:(i + 1) * P], t_ps)
                    for p8 in range(8):
                        t_ps = at_ps.tile([P, P], bf16, tag="sc", bufs=3,
                                          name="t_ps")
                        nc.tensor.transpose(t_ps[0:64, :], kn[:, 2 * p8, :],
                                            ident_bf)
                        nc.tensor.transpose(t_ps[64:128, :],
                                            kn[:, 2 * p8 + 1, :], ident_bf)
                        nc.vector.tensor_copy(kTd[:, p8 * P:(p8 + 1) * P],
                                              t_ps)
                    if pending_proj is not None:
                        pending_ln = emit_proj_ln(pending_proj)
                        pending_proj = None
                    oT_sb = outT_pool.tile([64, S], f32, tag="oTsb",
                                           bufs=2, name=f"oTsb{mi}")
                    LAG = 4
                    for gw in range(4):          # 512-wide q windows
                        Ws = 512 * gw
                        npairs = 2 * gw + 2
                        oTw = ot_ps.tile([65, 512], f32, tag="oT",
                                         name=f"oT{mi}_{gw}")
                        pend = []

                        def pop_av(half):
                            # one AV matmul (one half of a pending pair) so
                            # consecutive PE matmuls alternate PSUM banks
                            # (score bank vs oT bank) and fill/drain overlap.
                            if not pend:
                                return
                            p, a_sb, s0, s1 = pend[0]
                            if half == 0:
                                nc.tensor.matmul(
                                    oTw[:, s0:512], lhsT=v_sb[:, 2 * p, :],
                                    rhs=a_sb[:, s0:512],
                                    start=(p == 0), stop=False)
                            else:
                                nc.tensor.matmul(
                                    oTw[:, s1:512], lhsT=v_sb[:, 2 * p + 1, :],
                                    rhs=a_sb[:, 512 + s1:1024],
                                    start=False, stop=(p == npairs - 1))
                                pend.pop(0)

                        for p in range(npairs):
                            j0, j1 = 2 * p, 2 * p + 1
                            s0 = max(128 * j0 - Ws, 0)
                            s1 = max(128 * j1 - Ws, 0)
                            d0 = 128 * j0 >= Ws      # block straddles diagonal
                            d1 = 128 * j1 >= Ws
                            sc_ps = at_ps.tile([P, 1024], f32, tag="sc",
                                               bufs=3, name=f"sc{mi}_{gw}_{p}")
                            a_sb = at_sb.tile([P, 1024], bf16, tag="a",
                                              bufs=6, name=f"a{mi}_{gw}_{p}")
                            ready = len(pend) > LAG
                            nc.tensor.matmul(
                                sc_ps[:, s0:512],
                                lhsT=kTd[0:64, p * P:(p + 1) * P],
                                rhs=qTb[0:64, Ws + s0:Ws + 512],
                                start=True, stop=not d0)
                            if d0:
                                nc.tensor.matmul(
                                    sc_ps[:, s0:s0 + P], lhsT=ident_bf,
                                    rhs=trin, start=False, stop=True,
                                    skip_group_check=True)
                            if ready:
                                pop_av(0)
                            nc.tensor.matmul(
                                sc_ps[:, 512 + s1:1024],
                                lhsT=kTd[64:128, p * P:(p + 1) * P],
                                rhs=qTb[64:128, Ws + s1:Ws + 512],
                                start=True, stop=not d1)
                            if d1:
                                nc.tensor.matmul(
                                    sc_ps[:, 512 + s1:512 + s1 + P],
                                    lhsT=ident_bf, rhs=trin,
                                    start=False, stop=True,
                                    skip_group_check=True)
                            if ready:
                                pop_av(1)
                            nc.scalar.activation(a_sb, sc_ps, AF.Exp,
                                                 scale=float(INV_SCALE))
                            pend.append((p, a_sb, s0, s1))
                        while pend:
                            pop_av(0)
                            pop_av(1)
                        # normalize this window; only the partition-broadcast
                        # runs on gpsimd.
                        dn = at_sb.tile([1, 512], f32, tag="dn", bufs=4,
                                        name=f"dn{mi}_{gw}")
                        nc.vector.tensor_copy(dn, oTw[64:65, :])
                        nc.vector.reciprocal_approx_fast(dn, dn)
                        bc = at_sb.tile([64, 512], f32, tag="bc", bufs=4,
                                        name=f"bc{mi}_{gw}")
                        nc.gpsimd.partition_broadcast(bc, dn)
                        nc.vector.tensor_mul(oT_sb[:, Ws:Ws + 512],
                                             oTw[0:64, :], bc)
                        if gw == 1 and pending_ln is not None:
                            emit_lnT(pending_ln)
                            pending_ln = None
                    hT = head_sb.tile([P, 8, P], bf16, tag="hT",
                                      name=f"hT{mi}")
                    oT_r = oT_sb.rearrange("d (t a) -> d a t", a=16)
                    for kc in range(8):
                        for ah in range(2):
                            nc.gpsimd.tensor_copy(
                                hT[64 * ah:64 * ah + 64, kc, :],
                                oT_r[:, 2 * kc + ah, :])
                    pending_proj = (mi, hT)
                emit_lnT(emit_proj_ln(pending_proj))

            outT_cm.__exit__(None, None, None)

            # ---------------- Phase D: FFN ----------------
            # h1T computed directly: lhsT = w1 block (e,f), rhs = lnT over all
            # four m-tiles (e, 4*128 tokens) -> h1T (f, tokens). No h1
            # transposes needed, and h1T slices feed w2 as lhsT directly.
            with tc.tile_pool(name="ff_ps", bufs=4, space="PSUM") as ff_ps, \
                 tc.tile_pool(name="fo_ps", bufs=4, space="PSUM") as fo_ps, \
                 tc.tile_pool(name="ff_sb", bufs=1) as ff_sb, \
                 tc.tile_pool(name="wf_sb", bufs=3) as wf_sb, \
                 tc.tile_pool(name="w2_sb", bufs=4) as w2_sb, \
                 tc.tile_pool(name="o_sb", bufs=2) as o_pool:
                h1T = ff_sb.tile([P, 32, 4, P], fp16)   # (f-part, fc, token)
                w2_tiles = {}

                def w2_load(ns_i, kb):
                    w_sb = w2_sb.tile([P, 8, 512], fp16, tag="w2s",
                                      name=f"w2_{ns_i}_{kb}")
                    nc.scalar.dma_start(w_sb, w2[ns_i, kb])
                    w2_tiles[(ns_i, kb)] = w_sb

                for fcp in range(16):
                    w_sb = wf_sb.tile([P, 2, 8, P], fp16, tag="w1s",
                                      name=f"w1_{fcp}")
                    nc.scalar.dma_start(w_sb, w1e[fcp])
                    if fcp == 11:
                        w2_load(0, 0)
                    elif fcp == 13:
                        w2_load(0, 1)
                    facc = [ff_ps.tile([P, 512], f32, tag="facc",
                                       name=f"fa{2 * fcp + fl}")
                            for fl in range(2)]
                    if not zero_bias:
                        for fl in range(2):
                            fc = 2 * fcp + fl
                            nc.tensor.matmul(
                                facc[fl],
                                lhsT=b1_b[:, 128 * fc:128 * (fc + 1)],
                                rhs=ones_h512, start=True, stop=False)
                    for kc in range(8):
                        for fl in range(2):
                            nc.tensor.matmul(
                                facc[fl], lhsT=w_sb[:, fl, kc, :],
                                rhs=lnT[:, kc, :, :],
                                start=(zero_bias and kc == 0),
                                stop=(kc == 7))
                    for fl in range(2):
                        nc.vector.tensor_relu(h1T[:, 2 * fcp + fl, :, :],
                                              facc[fl])
                for ns_i in range(2):
                    ns = slice(ns_i * 512, (ns_i + 1) * 512)
                    acc = [fo_ps.tile([P, 512], f32, tag="oacc",
                                      name=f"oa{ns_i}_{mi}") for mi in range(4)]
                    if not zero_bias:
                        for mi in range(4):
                            nc.tensor.matmul(acc[mi], lhsT=ones_h,
                                             rhs=b2_b[:, ns],
                                             start=True, stop=False)
                    for kb in range(4):
                        if (ns_i, kb) not in w2_tiles:
                            w2_load(ns_i, kb)
                        w_sb = w2_tiles[(ns_i, kb)]
                        for kc8 in range(8):
                            kc = 8 * kb + kc8
                            for mi in range(4):
                                nc.tensor.matmul(acc[mi],
                                                 lhsT=h1T[:, kc, mi, :],
                                                 rhs=w_sb[:, kc8, :],
                                                 start=(zero_bias and kc == 0),
                                                 stop=(kc == 31))
                    for mi, (b, hh) in enumerate(mlist):
                        o_sb = o_pool.tile([P, 512], f32, tag="o",
                                           name=f"o{ns_i}_{mi}")
                        nc.vector.tensor_copy(o_sb, acc[mi])
                        nc.sync.dma_start(out[b, hh, :, ns], o_sb)

            ln_pool_cm.__exit__(None, None, None)

    nc.compile()
    return nc


def _get_nc(zero_bias=False):
    key = ("nc", zero_bias)
    if key not in _cached:
        _cached[key] = _build(zero_bias=zero_bias)
    return _cached[key]


def _make_in_maps(inputs):
    import ml_dtypes
    x = np.ascontiguousarray(np.asarray(inputs["x"], dtype=np.float32))
    w_qkv = np.ascontiguousarray(np.asarray(inputs["w_qkv"], dtype=np.float32))
    b_qkv = np.asarray(inputs["b_qkv"], dtype=np.float32).reshape(1, ROW)
    w_proj = np.ascontiguousarray(np.asarray(inputs["w_proj"], dtype=np.float32))
    b_proj = np.asarray(inputs["b_proj"], dtype=np.float32).reshape(1, E)
    ln_g = np.asarray(inputs["ln_g"], dtype=np.float32)
    ln_b = np.asarray(inputs["ln_b"], dtype=np.float32)
    w1 = np.asarray(inputs["w1"], dtype=np.float32)
    b1 = np.asarray(inputs["b1"], dtype=np.float32)
    w2 = np.ascontiguousarray(np.asarray(inputs["w2"], dtype=np.float32))
    b2 = np.asarray(inputs["b2"], dtype=np.float32).reshape(1, E)

    w1e = (ln_g[:, None] * w1).astype(np.float32)
    b1e = (b1 + ln_b @ w1).reshape(1, FF).astype(np.float32)

    # wqkv [3, 128, 8, 1024]: [n2, p, kc, f] = w_qkv[kc*128+p, n2*1024+f]
    w_qkv_t = np.ascontiguousarray(
        w_qkv.reshape(8, P, 3, 1024).transpose(2, 1, 0, 3)
    ).astype(ml_dtypes.bfloat16)
    # wproj [2, 128, 8, 512]
    w_proj_t = np.ascontiguousarray(
        w_proj.reshape(8, P, 2, 512).transpose(2, 1, 0, 3)
    ).astype(ml_dtypes.bfloat16)
    # w1e [16, 128, 2, 8, 128]: [fcp, p, fl, kc, f] = w1e[kc*128+p, (2*fcp+fl)*128+f]
    w1e_t = np.ascontiguousarray(
        w1e.reshape(8, P, 16, 2, P).transpose(2, 1, 3, 0, 4)
    ).astype(np.float16)
    # w2 [2, 4, 128, 8, 512]: [ns, kb, p, kc8, f] = w2[(kb*8+kc8)*128+p, ns*512+f]
    w2_t = np.ascontiguousarray(
        w2.reshape(4, 8, P, 2, 512).transpose(3, 0, 2, 1, 4)
    ).astype(np.float16)

    ones_host = np.ones((P, 130), np.float32)
    # additive causal mask for diagonal tiles: 0 where q >= k, -30000 where
    # q < k (exp(-30000/32) == 0)
    triu_host = ((np.triu(np.ones((P, P))) - 1.0) *
                 30000.0).astype(ml_dtypes.bfloat16)
    b_qkv_b = b_qkv.astype(ml_dtypes.bfloat16)
    b_proj_b = b_proj.astype(ml_dtypes.bfloat16)
    b1e_h = b1e.astype(np.float16)
    b2_h = b2.astype(np.float16)
    in_maps = []
    for c in range(NCORES):
        # xq [B, 3, 128, 8, 88]: pre-transposed x rows per slot:
        # [b, t, p, kc, r] = x[b, T0+r, kc*128+p]
        xq = np.zeros((B, 3, P, 8, 88), ml_dtypes.bfloat16)
        offs = np.zeros((1, 4), np.uint32)
        for t in range(3):
            start = (16 * t + 2 * c) * BLK
            T0 = start // ROW
            offs[0, t] = ROW - (start - T0 * ROW)
            n = min(88, S - T0)
            for b in range(B):
                xt = x[b, T0:T0 + n].T.reshape(8, P, n)   # [kc, p, r]
                xq[b, t, :, :, :n] = xt.transpose(1, 0, 2)
        # b_proj folded into the residual input
        xr = np.zeros((B, 2, P, E), np.float32)
        for hh in range(2):
            h_ = 2 * c + hh
            for b in range(B):
                xr[b, hh] = x[b, P * h_:P * (h_ + 1)] + b_proj
        in_maps.append({
            "xq": xq, "xr": xr, "offs": offs,
            "ones": ones_host, "triu": triu_host,
            "wqkv": w_qkv_t, "bqkv": b_qkv_b, "wproj": w_proj_t,
            "bproj": b_proj_b,
            "w1e": w1e_t, "b1e": b1e_h, "w2": w2_t, "b2": b2_h,
        })
    return in_maps


def _run(inputs, trace=False, trace_cores=None):
    import sys
    if "/opt/trn_rl_repo" not in sys.path:
        sys.path.insert(0, "/opt/trn_rl_repo")
    from concourse.bass_utils import run_bass_kernel_spmd
    zero_bias = bool(
        not np.any(np.asarray(inputs["b_qkv"]))
        and not np.any(np.asarray(inputs["b1"]))
        and not np.any(np.asarray(inputs["ln_b"]))
        and not np.any(np.asarray(inputs["b2"])))
    nc = _get_nc(zero_bias=zero_bias)
    in_maps = _make_in_maps(inputs)
    kwargs = {}
    if trace:
        kwargs["trace"] = True
        if trace_cores is not None:
            kwargs["trace_cores"] = trace_cores
    res = run_bass_kernel_spmd(nc, in_maps, list(range(NCORES)), **kwargs)
    full = np.zeros((B, S, E), np.float32)
    for c in range(NCORES):
        o = res.results[c]["out"]
        for hh in range(2):
            h_ = 2 * c + hh
            for b in range(B):
                full[b, P * h_:P * (h_ + 1)] = o[b, hh]
    return full, res


def kernel(**inputs) -> np.ndarray:
    import sys
    if "/opt/trn_rl_repo" not in sys.path:
        sys.path.insert(0, "/opt/trn_rl_repo")
    full, _ = _run(inputs)
    return full
